# revision 1
# baseline (speedup 1.0000x reference)
# BitNet GQA attention block on 8 Trainium2 NeuronCores.
#
# Sharding: data parallel over sequence (256 tokens/core). K/V are computed
# per-core for the local tokens, RoPE'd, then AllGathered so every core can
# run full (non-causal) attention for its own query tokens. Projections run
# as integer-exact bf16 matmuls (8-bit quantized activations are integers
# <=127, ternary weights are -1/0/1 -- both exact in bf16; PSUM accumulates
# in fp32 and |dot| < 2^24 so results are exact). Attention matmuls use
# float32r (full PE rate at free-dim >= 256).
import math

import numpy as np

import concourse.bacc as bacc
import concourse.bass as bass
import concourse.bass_isa as bass_isa
import concourse.mybir as mybir
import concourse.tile as tile

DT = mybir.dt
AF = mybir.ActivationFunctionType
ALU = mybir.AluOpType
AX = mybir.AxisListType

H = 4096
QH, KVH, D = 32, 8, 128     # query heads, kv heads, head dim
HB = H // 128               # 32 hidden blocks
NREP = QH // KVH
ROUND_C = 12582912.0        # 1.5 * 2**23: fp32 add forces round-to-nearest-even int
LN_EPS = 1e-5
QB = 127.0
SM_SCALE = 1.0 / math.sqrt(128.0)


def build(n_cores=8, s_pc=256, stub_collectives=False, body_reps=1,
          skip_gb=False):
    """Build the SPMD Bass program (identical on all cores; per-core data via inputs)."""
    NT = s_pc // 128            # token tiles per core
    S = s_pc * n_cores
    KT = S // 128               # key-token tiles after gather
    f32, bf16, f32r = DT.float32, DT.bfloat16, DT.float32r

    nc = bacc.Bacc("TRN2", target_bir_lowering=False, debug=False, num_devices=n_cores)

    x_d = nc.dram_tensor("x", [s_pc, H], f32, kind="ExternalInput").ap()
    g_d = nc.dram_tensor("lng", [1, H], f32, kind="ExternalInput").ap()
    b_d = nc.dram_tensor("lnb", [1, H], f32, kind="ExternalInput").ap()
    cos_d = nc.dram_tensor("cosT", [D, s_pc], f32, kind="ExternalInput").ap()
    sin_d = nc.dram_tensor("sinTs", [D, s_pc], f32, kind="ExternalInput").ap()
    wq_d = nc.dram_tensor("wqt", [128, QH, HB, 128], bf16, kind="ExternalInput").ap()
    wk_d = nc.dram_tensor("wkt", [128, KVH, HB, 128], bf16, kind="ExternalInput").ap()
    wv_d = nc.dram_tensor("wvt", [128, HB, KVH * D], bf16, kind="ExternalInput").ap()
    wo_d = nc.dram_tensor("wot", [128, HB, HB, 128], bf16, kind="ExternalInput").ap()
    sc_d = nc.dram_tensor("wscal", [1, 4], f32, kind="ExternalInput").ap()
    onesr_d = nc.dram_tensor("onesr", [128, 1], f32r, kind="ExternalInput").ap()
    y_d = nc.dram_tensor("yT", [H, s_pc], f32, kind="ExternalOutput").ap()

    with tile.TileContext(nc) as tc:
        for rep in range(body_reps):
            _body(nc, tc, n_cores, s_pc, NT, KT,
                  x_d, g_d, b_d, cos_d, sin_d, wq_d, wk_d, wv_d, wo_d, sc_d,
                  onesr_d, y_d, stub_collectives, pfx=f"r{rep}_",
                  skip_gb=skip_gb)
    nc.compile()
    return nc


def _body(nc, tc, n_cores, s_pc, NT, KT,
          x_d, g_d, b_d, cos_d, sin_d, wq_d, wk_d, wv_d, wo_d, sc_d,
          onesr_d, y_d, stub_collectives=False, pfx="", skip_gb=False):
    f32, bf16, f32r = DT.float32, DT.bfloat16, DT.float32r
    sync, vec, act, pe, gp = nc.sync, nc.vector, nc.scalar, nc.tensor, nc.gpsimd

    from contextlib import ExitStack

    def bcast_row(psb_pool, ones1, row, out_sb, n, name):
        """Replicate [1, n] row across 128 partitions via K=1 fp32 matmul
        (exact: 1.0 * a) then copy PSUM->SBUF."""
        for i, n0 in enumerate(range(0, n, 512)):
            nn = min(512, n - n0)
            ps = psb_pool.tile([128, 512], f32, name=f"{name}_ps{i}", tag="psb")
            pe.matmul(ps[:, 0:nn], ones1, row[:, n0:n0 + nn],
                      start=True, stop=True)
            vec.tensor_copy(out_sb[:, n0:n0 + nn], ps[:, 0:nn])

    def bcast_from_dram(psb_pool, row_pool, ones1, dram_row, out_sb, n, name):
        """Like bcast_row but streams the source row from DRAM in [1, 512]
        chunks (avoids a [1, n] SBUF tile, which costs n*4 bytes on every
        partition)."""
        for i, n0 in enumerate(range(0, n, 512)):
            nn = min(512, n - n0)
            rt = row_pool.tile([1, 512], f32, name=f"{name}_row{i}", tag="brow")
            sync.dma_start(rt[:, 0:nn], dram_row[:, n0:n0 + nn])
            ps = psb_pool.tile([128, 512], f32, name=f"{name}_ps{i}", tag="psb")
            pe.matmul(ps[:, 0:nn], ones1, rt[:, 0:nn], start=True, stop=True)
            vec.tensor_copy(out_sb[:, n0:n0 + nn], ps[:, 0:nn])

    es = ExitStack()
    with es:
        # ---------------- long-lived pools ----------------
        constp = es.enter_context(tc.tile_pool(name=pfx + "constp", bufs=1))
        dramp = es.enter_context(tc.tile_pool(name=pfx + "dramp", bufs=1, space="DRAM"))
        xTp = es.enter_context(tc.tile_pool(name=pfx + "xTp", bufs=1))
        qTp = es.enter_context(tc.tile_pool(name=pfx + "qTp", bufs=1))
        aop = es.enter_context(tc.tile_pool(name=pfx + "aop", bufs=1))

        cosS = constp.tile([D, s_pc], f32, name="cosS", tag="cosS")
        sinS = constp.tile([D, s_pc], f32, name="sinS", tag="sinS")
        sync.dma_start(cosS, cos_d)
        sync.dma_start(sinS, sin_d)
        ones1 = constp.tile([1, 128], f32, name="ones1", tag="ones1")
        vec.memset(ones1, 1.0)
        scal_sb = constp.tile([128, 4], f32, name="scal_sb", tag="scal_sb")
        scal_row = constp.tile([1, 4], f32, name="scal_row", tag="scal_row")
        sync.dma_start(scal_row, sc_d)
        sw_q, sw_k, sw_v, sw_o = (scal_sb[:, i:i + 1] for i in range(4))
        ones_sb = constp.tile([128, 1], f32r, name="ones_sb", tag="ones_sb")
        sync.dma_start(ones_sb, onesr_d)

        # quantized+transposed activations [hid, tok] as bf16 integers
        xT = xTp.tile([128, HB, s_pc], bf16, name="xT", tag="xT")
        # per-token dequant scale r_i = clip(absmax,1e-5)/127, replicated on all partitions
        R = xTp.tile([128, s_pc], f32, name="R", tag="R")
        r_dram = dramp.tile([1, s_pc], f32, name="r_dram", tag="r_dram")

        qTall = qTp.tile([128, QH, s_pc], f32r, name="qTall", tag="qTall")
        aoall = aop.tile([128, QH, s_pc], f32, name="aoall", tag="aoall")
        acc = aop.tile([128, s_pc], f32, name="acc", tag="acc")
        vec.memset(acc, 0.0)

        # collective buffers
        ksrc = dramp.tile([KVH, D, s_pc], f32r, name="ksrc", tag="ksrc")
        vsrc = dramp.tile([NT, 128, KVH * D], f32r, name="vsrc", tag="vsrc")
        kv_space = "Local" if stub_collectives else "Shared"
        KG = dramp.tile([n_cores, KVH, D, s_pc], f32r, name="KG", tag="KG",
                        addr_space=kv_space)
        VG = dramp.tile([n_cores, NT, 128, KVH * D], f32r, name="VG", tag="VG",
                        addr_space=kv_space)

        r_tiles = []

        # per-token scale tiles (partition layout) -- live into phase 2
        for t in range(NT):
            r_t = constp.tile([128, 1], f32, name=f"r_{t}", tag=f"r_{t}")
            r_tiles.append(r_t)

        # ---------------- phase 1: layernorm + act quant ----------------
        with tc.tile_pool(name=pfx + "lnp", bufs=2) as lnp, \
             tc.tile_pool(name=pfx + "gbp", bufs=1) as gbp, \
             tc.tile_pool(name=pfx + "statp", bufs=1) as statp, \
             tc.tile_pool(name=pfx + "psb1", bufs=2, space="PSUM") as psb1, \
             tc.tile_pool(name=pfx + "xqp", bufs=2) as xqp:
            if not skip_gb:
                Gt = gbp.tile([128, H], f32, name="Gt", tag="Gt")
                Bt = gbp.tile([128, H], f32, name="Bt", tag="Bt")
                bcast_from_dram(psb1, gbp, ones1, g_d, Gt, H, "g")
                bcast_from_dram(psb1, gbp, ones1, b_d, Bt, H, "b")
            bcast_row(psb1, ones1, scal_row, scal_sb, 4, "sc")

            for t in range(NT):
                xs = lnp.tile([128, H], f32, name=f"xs{t}", tag="xs")
                scr = lnp.tile([128, H], f32, name=f"scr{t}", tag="scr")
                sync.dma_start(xs, x_d[t * 128:(t + 1) * 128, :])

                nsum = statp.tile([128, 1], f32, name=f"nsum{t}", tag=f"nsum{t}")
                vec.tensor_reduce(nsum, xs, axis=AX.X, op=ALU.add, negate=True)
                nmu = statp.tile([128, 1], f32, name=f"nmu{t}", tag=f"nmu{t}")
                vec.tensor_scalar_mul(nmu, nsum, 1.0 / H)
                sumsq = statp.tile([128, 1], f32, name=f"sumsq{t}", tag=f"sumsq{t}")
                act.activation(scr, xs, AF.Square, bias=nmu, scale=1.0,
                               accum_out=sumsq)
                varv = statp.tile([128, 1], f32, name=f"varv{t}", tag=f"varv{t}")
                vec.tensor_scalar(varv, sumsq, 1.0 / H, LN_EPS, ALU.mult, ALU.add)
                stdv = statp.tile([128, 1], f32, name=f"stdv{t}", tag=f"stdv{t}")
                act.activation(stdv, varv, AF.Sqrt)
                rstd = statp.tile([128, 1], f32, name=f"rstd{t}", tag=f"rstd{t}")
                vec.reciprocal(rstd, stdv)
                nmr = statp.tile([128, 1], f32, name=f"nmr{t}", tag=f"nmr{t}")
                vec.tensor_mul(nmr, nmu, rstd)
                # normed = x*rstd + (-mu*rstd), then *g + b (in place)
                act.activation(xs, xs, AF.Identity, bias=nmr, scale=rstd)
                if not skip_gb:
                    vec.tensor_mul(xs, xs, Gt)
                    vec.tensor_add(xs, xs, Bt)

                am = statp.tile([128, 1], f32, name=f"am{t}", tag=f"am{t}")
                vec.tensor_reduce(am, xs, axis=AX.X, op=ALU.max,
                                  apply_absolute_value=True)
                amc = statp.tile([128, 1], f32, name=f"amc{t}", tag=f"amc{t}")
                vec.tensor_scalar_max(amc, am, 1e-5)
                r_t = r_tiles[t]
                vec.tensor_scalar_mul(r_t, amc, 1.0 / QB)
                inv = statp.tile([128, 1], f32, name=f"inv{t}", tag=f"inv{t}")
                vec.reciprocal(inv, amc)
                scq = statp.tile([128, 1], f32, name=f"scq{t}", tag=f"scq{t}")
                vec.tensor_scalar_mul(scq, inv, QB)

                # n = round(normed * scq), exact via +C trick; write as bf16 ints
                vec.tensor_scalar(scr, xs, scq, ROUND_C, ALU.mult, ALU.add)
                xq = xqp.tile([128, H], bf16, name=f"xq{t}", tag="xq")
                vec.tensor_scalar_add(xq, scr, -ROUND_C)

                # transpose into [hid, tok] layout (DMA xbar transpose, bf16)
                for h in range(HB):
                    sync.dma_start(xT[:, h, t * 128:(t + 1) * 128],
                                   xq[:, h * 128:(h + 1) * 128], transpose=True)
                # export per-token scale
                sync.dma_start(r_dram[0, t * 128:(t + 1) * 128], r_t[:, 0])

            r_row = constp.tile([1, s_pc], f32, name="r_row", tag="r_row")
            sync.dma_start(r_row, r_dram[:])
            bcast_row(psb1, ones1, r_row, R, s_pc, "r")

        # ---------------- phase 2: K,V projections + rope + gather ----------------
        with tc.tile_pool(name=pfx + "wkvp", bufs=3) as wkvp, \
             tc.tile_pool(name=pfx + "pskv", bufs=2, space="PSUM") as pskv, \
             tc.tile_pool(name=pfx + "psv", bufs=1, space="PSUM") as psvp, \
             tc.tile_pool(name=pfx + "kdrp", bufs=2) as kdrp:
            # K projection: kT[feat, tok] per kv head
            for f in range(KVH):
                wk_sb = wkvp.tile([128, HB, 128], bf16, name=f"wk{f}", tag="wkv")
                sync.dma_start(wk_sb, wk_d[:, f, :, :])
                ps = pskv.tile([128, s_pc], f32, name=f"psk{f}", tag="pskv")
                for k in range(HB):
                    pe.matmul(ps, wk_sb[:, k, :], xT[:, k, :],
                              start=(k == 0), stop=(k == HB - 1))
                kdr = kdrp.tile([128, s_pc], f32, name=f"kdr{f}", tag="kdr")
                vec.scalar_tensor_tensor(kdr, ps, sw_k, R, op0=ALU.mult,
                                         op1=ALU.mult)
                # rope
                rot = kdrp.tile([128, s_pc], f32, name=f"krot{f}", tag="krot")
                sync.dma_start(rot[0:64, :], kdr[64:128, :])
                sync.dma_start(rot[64:128, :], kdr[0:64, :])
                vec.tensor_mul(rot, rot, sinS)
                kcos = kdrp.tile([128, s_pc], f32, name=f"kcos{f}", tag="kcos")
                vec.tensor_mul(kcos, kdr, cosS)
                krp = kdrp.tile([128, s_pc], f32r, name=f"krp{f}", tag="krp")
                vec.tensor_add(krp, kcos, rot)
                sync.dma_start(ksrc[f], krp)

            gp.collective_compute(
                "AllGather", ALU.bypass,
                replica_groups=[list(range(n_cores))],
                ins=[ksrc.opt()], outs=[KG.opt()]) \
                if not stub_collectives else [
                    sync.dma_start(KG[r], ksrc) for r in range(n_cores)]

            # V projection: v[tok, feat] (x^T as stationary, weights moving)
            psvs = [psvp.tile([128, KVH * D], f32, name=f"psv{t}",
                              tag=f"psv{t}") for t in range(NT)]
            for k in range(HB):
                wv_sb = wkvp.tile([128, KVH * D], bf16, name=f"wv{k}",
                                  tag="wv")
                sync.dma_start(wv_sb, wv_d[:, k, :])
                for t in range(NT):
                    for n0 in (0, 512):
                        pe.matmul(psvs[t][:, n0:n0 + 512],
                                  xT[:, k, t * 128:(t + 1) * 128],
                                  wv_sb[:, n0:n0 + 512],
                                  start=(k == 0), stop=(k == HB - 1))
            for t in range(NT):
                vdr = kdrp.tile([128, KVH * D], f32r, name=f"vdr{t}", tag="vdr")
                vec.tensor_scalar(vdr, psvs[t], r_tiles[t], sw_v,
                                  ALU.mult, ALU.mult)
                sync.dma_start(vsrc[t], vdr)

            if stub_collectives:
                for r in range(n_cores):
                    sync.dma_start(VG[r], vsrc)
            else:
                gp.collective_compute(
                    "AllGather", ALU.bypass,
                    replica_groups=[list(range(n_cores))],
                    ins=[vsrc.opt()], outs=[VG.opt()])

        # ---------------- phase 3: Q projection + rope ----------------
        with tc.tile_pool(name=pfx + "wqp", bufs=3) as wqp, \
             tc.tile_pool(name=pfx + "psq", bufs=4, space="PSUM") as psqp, \
             tc.tile_pool(name=pfx + "qdrp", bufs=2) as qdrp:
            for f in range(QH):
                wq_sb = wqp.tile([128, HB, 128], bf16, name=f"wq{f}", tag="wq")
                sync.dma_start(wq_sb, wq_d[:, f, :, :])
                ps = psqp.tile([128, s_pc], f32, name=f"psq{f}", tag="psq")
                for k in range(HB):
                    pe.matmul(ps, wq_sb[:, k, :], xT[:, k, :],
                              start=(k == 0), stop=(k == HB - 1))
                qdr = qdrp.tile([128, s_pc], f32, name=f"qdr{f}", tag="qdr")
                vec.scalar_tensor_tensor(qdr, ps, sw_q, R, op0=ALU.mult,
                                         op1=ALU.mult)
                rot = qdrp.tile([128, s_pc], f32, name=f"qrot{f}", tag="qrot")
                sync.dma_start(rot[0:64, :], qdr[64:128, :])
                sync.dma_start(rot[64:128, :], qdr[0:64, :])
                vec.tensor_mul(rot, rot, sinS)
                qcos = qdrp.tile([128, s_pc], f32, name=f"qcos{f}", tag="qcos")
                vec.tensor_mul(qcos, qdr, cosS)
                vec.tensor_add(qTall[:, f, :], qcos, rot)

        # ---------------- phase 4: attention ----------------
        with tc.tile_pool(name=pfx + "kgp", bufs=2) as kgp, \
             tc.tile_pool(name=pfx + "vgp", bufs=2) as vgp, \
             tc.tile_pool(name=pfx + "ep", bufs=2) as ep, \
             tc.tile_pool(name=pfx + "pss", bufs=2, space="PSUM") as pssp, \
             tc.tile_pool(name=pfx + "psd", bufs=1, space="PSUM") as psdp, \
             tc.tile_pool(name=pfx + "pso", bufs=2, space="PSUM") as psop, \
             tc.tile_pool(name=pfx + "psb4", bufs=1, space="PSUM") as psb4, \
             tc.tile_pool(name=pfx + "dnp", bufs=2) as dnp:
            for g in range(KVH):
                kg_sb = kgp.tile([128, n_cores, s_pc], f32r, name=f"kg{g}",
                                 tag="kg")
                sync.dma_start(kg_sb, KG[:, g, :, :].rearrange("r d t -> d r t"))
                vg_sb = vgp.tile([128, KT, 128], f32r, name=f"vg{g}", tag="vg")
                sync.dma_start(
                    vg_sb,
                    VG.rearrange("r t p f -> p (r t) f")[:, :,
                                                         g * 128:(g + 1) * 128])
                for j in range(NREP):
                    hq = g * NREP + j
                    qh_ap = qTall[:, hq, :]
                    eT = ep.tile([128, KT, s_pc], f32r, name=f"eT{g}_{j}",
                                 tag="eT")
                    # scores in groups of 4 key-tiles per PSUM tile so each
                    # Exp covers [128, 1024] (amortizes PSUM access latency)
                    GRP = 4
                    for kt0 in range(0, KT, GRP):
                        pss = pssp.tile([128, GRP * s_pc], f32,
                                        name=f"pss{g}_{j}_{kt0}", tag="pss")
                        for u in range(GRP):
                            kt = kt0 + u
                            pe.matmul(pss[:, u * s_pc:(u + 1) * s_pc],
                                      kg_sb[:, kt // NT,
                                            (kt % NT) * 128:
                                            (kt % NT) * 128 + 128],
                                      qh_ap, start=True, stop=True)
                        act.activation(
                            eT[:, kt0:kt0 + GRP, :].rearrange("p a b -> p (a b)"),
                            pss, AF.Exp, scale=SM_SCALE)
                    # denominators: ones^T @ eT accumulated over key tiles
                    psd = psdp.tile([1, s_pc], f32, name=f"psd{g}_{j}",
                                    tag="psd")
                    for kt in range(KT):
                        pe.matmul(psd, ones_sb,
                                  eT[:, kt, :],
                                  start=(kt == 0), stop=(kt == KT - 1))
                    # attention output (unnormalized): v^T as stationary
                    pso = psop.tile([128, s_pc], f32, name=f"pso{g}_{j}",
                                    tag="pso")
                    for kt in range(KT):
                        pe.matmul(pso, vg_sb[:, kt, :],
                                  eT[:, kt, :],
                                  start=(kt == 0), stop=(kt == KT - 1))
                    # normalize by softmax denominator
                    dnrow = dnp.tile([1, s_pc], f32, name=f"dnrow{g}_{j}",
                                     tag="dnrow")
                    vec.tensor_copy(dnrow, psd)
                    dps = psb4.tile([128, s_pc], f32, name=f"dps{g}_{j}",
                                    tag="dps")
                    pe.matmul(dps, ones1, dnrow, start=True, stop=True)
                    dnr = dnp.tile([128, s_pc], f32, name=f"dnr{g}_{j}",
                                   tag="dnr")
                    vec.reciprocal(dnr, dps)
                    vec.tensor_tensor(aoall[:, hq, :], pso, dnr, ALU.mult)
                    vec.tensor_tensor(acc, acc, aoall[:, hq, :], ALU.max)
                    vec.scalar_tensor_tensor(acc, aoall[:, hq, :], -1.0, acc,
                                             op0=ALU.mult, op1=ALU.max)

        # ---------------- phase 5: re-quant + O projection ----------------
        with tc.tile_pool(name=pfx + "q2p", bufs=1) as q2p, \
             tc.tile_pool(name=pfx + "t1p", bufs=2) as t1p, \
             tc.tile_pool(name=pfx + "wop", bufs=3) as wop, \
             tc.tile_pool(name=pfx + "psy", bufs=4, space="PSUM") as psyp, \
             tc.tile_pool(name=pfx + "psb5", bufs=1, space="PSUM") as psb5, \
             tc.tile_pool(name=pfx + "yp", bufs=2) as yp:
            # per-token absmax over the partition dim: halving tree
            # (acc is already elementwise |.|-accumulated, all >= 0)
            tmp = q2p.tile([64, s_pc], f32, name="tmphalf", tag="tmphalf")
            cur = 128
            while cur > 1:
                h = cur // 2
                sync.dma_start(tmp[0:h, :], acc[h:cur, :])
                vec.tensor_tensor(acc[0:h, :], acc[0:h, :], tmp[0:h, :],
                                  ALU.max)
                cur = h
            r2row = q2p.tile([1, s_pc], f32, name="r2row", tag="r2row")
            vec.tensor_scalar(r2row, acc[0:1, :], 1e-5, 1.0 / QB,
                              ALU.max, ALU.mult)
            R2 = q2p.tile([128, s_pc], f32, name="R2", tag="R2")
            bcast_row(psb5, ones1, r2row, R2, s_pc, "r2")
            sc2 = q2p.tile([128, s_pc], f32, name="sc2", tag="sc2")
            vec.reciprocal(sc2, R2)
            ao2 = q2p.tile([128, HB, s_pc], bf16, name="ao2", tag="ao2")
            for f in range(QH):
                t1 = t1p.tile([128, s_pc], f32, name=f"t1_{f}", tag="t1")
                vec.tensor_tensor(t1, aoall[:, f, :], sc2, ALU.mult)
                vec.tensor_scalar(ao2[:, f, :], t1, ROUND_C, -ROUND_C,
                                  ALU.add, ALU.add)

            for f in range(HB):
                wo_sb = wop.tile([128, HB, 128], bf16, name=f"wo{f}", tag="wo")
                sync.dma_start(wo_sb, wo_d[:, f, :, :])
                ps = psyp.tile([128, s_pc], f32, name=f"psy{f}", tag="psy")
                for k in range(HB):
                    pe.matmul(ps, wo_sb[:, k, :], ao2[:, k, :],
                              start=(k == 0), stop=(k == HB - 1))
                yT_sb = yp.tile([128, s_pc], f32, name=f"yT{f}", tag="yT")
                vec.scalar_tensor_tensor(yT_sb, ps, sw_o, R2, op0=ALU.mult,
                                         op1=ALU.mult)
                sync.dma_start(y_d[f * 128:(f + 1) * 128, :], yT_sb)


# ---------------------------------------------------------------------------
# host side
# ---------------------------------------------------------------------------

def _weight_quant_host(W):
    """Mimic reference _weight_quant: returns ternary m in {-1,0,1} and the
    effective dequant scale (1/scale) as fp32."""
    W = np.asarray(W, dtype=np.float32)
    mean_abs = np.float32(np.mean(np.abs(W), dtype=np.float64))
    clipped = np.maximum(mean_abs, np.float32(1e-5))
    scale = np.float32(1.0) / clipped
    m = np.clip(np.round(W * scale), -1.0, 1.0).astype(np.float32)
    sw = np.float32(1.0) / scale    # dequant scale applied after int matmul
    return m, float(sw)


def _prep_weights(Wq, Wk, Wv, Wo):
    mq, swq = _weight_quant_host(Wq)
    mk, swk = _weight_quant_host(Wk)
    mv, swv = _weight_quant_host(Wv)
    mo, swo = _weight_quant_host(Wo)

    def blocked(mT, fb):  # mT: [H, out] -> [128, fb, HB, 128]
        return np.ascontiguousarray(
            mT.reshape(HB, 128, fb, 128).transpose(1, 2, 0, 3)
        ).astype(np.float32)

    wqt = blocked(mq.T, QH)
    wkt = blocked(mk.T, KVH)
    wot = blocked(mo.T, HB)
    wvt = np.ascontiguousarray(
        mv.T.reshape(HB, 128, KVH * D).transpose(1, 0, 2)).astype(np.float32)
    import ml_dtypes
    tob = lambda a: a.astype(ml_dtypes.bfloat16)
    return (tob(wqt), tob(wkt), tob(wvt), tob(wot),
            np.array([[swq, swk, swv, swo]], dtype=np.float32))


def _rope_tables(S):
    inv = (1.0 / (10000.0 ** (np.arange(0, D, 2, dtype=np.float32)
                              / np.float32(D)))).astype(np.float32)
    pos = np.arange(S, dtype=np.float32)
    fr = pos[:, None] * inv[None, :]          # [S, 64]
    emb = np.concatenate([fr, fr], axis=1)    # [S, D]
    cosT = np.cos(emb).T.astype(np.float32).copy()   # [D, S]
    sinT = np.sin(emb).T.astype(np.float32).copy()
    sinT[0:64, :] *= -1.0                      # sign baked for rotate-half
    return cosT, sinT


def _in_maps(inputs, n_cores=8, s_pc=256):
    hs = np.asarray(inputs["hidden_states"], dtype=np.float32)
    ln_w = np.asarray(inputs["ln_w"], dtype=np.float32).reshape(1, H)
    ln_b = np.asarray(inputs["ln_b"], dtype=np.float32).reshape(1, H)
    wqt, wkt, wvt, wot, wscal = _prep_weights(
        inputs["Wq"], inputs["Wk"], inputs["Wv"], inputs["Wo"])
    S = hs.shape[1]
    cosT, sinT = _rope_tables(S)
    maps = []
    for c in range(n_cores):
        sl = slice(c * s_pc, (c + 1) * s_pc)
        maps.append({
            "x": np.ascontiguousarray(hs[0, sl, :]),
            "lng": ln_w, "lnb": ln_b,
            "cosT": np.ascontiguousarray(cosT[:, sl]),
            "sinTs": np.ascontiguousarray(sinT[:, sl]),
            "wqt": wqt, "wkt": wkt, "wvt": wvt, "wot": wot,
            "wscal": wscal,
            "onesr": np.ones((128, 1), dtype=np.float32),
        })
    return maps


_CACHED = {}


def _run(inputs, trace=False, n_cores=8, s_pc=256):
    from concourse.bass_utils import run_bass_kernel_spmd
    skip_gb = bool(
        np.allclose(np.asarray(inputs["ln_w"]), 1.0)
        and np.allclose(np.asarray(inputs["ln_b"]), 0.0))
    key = (n_cores, s_pc, skip_gb)
    if key not in _CACHED:
        _CACHED[key] = build(n_cores, s_pc, skip_gb=skip_gb)
    nc = _CACHED[key]
    maps = _in_maps(inputs, n_cores, s_pc)
    res = run_bass_kernel_spmd(nc, maps, list(range(n_cores)), trace=trace)
    parts = [res.results[c]["yT"].T for c in range(n_cores)]
    y = np.concatenate(parts, axis=0)[None, :, :].astype(np.float32)
    return y, res.exec_time_ns


def kernel(**inputs):
    y, _ = _run(inputs, trace=False)
    return y



# revision 57
# speedup vs baseline: 10.4181x; 10.4181x over previous
# BitNet GQA attention block on 8 Trainium2 NeuronCores.
#
# Sharding: data parallel over sequence (256 tokens/core). K/V are computed
# per-core for the local tokens, RoPE'd, then AllGathered so every core can
# run full (non-causal) attention for its own query tokens.
#
# Numerics: projections run as integer-exact matmuls -- 8-bit quantized
# activations are integers <=127 held in bf16, ternary weights are -1/0/1
# held in fp8e4m3 (both exact; mixed-dtype matmul accumulates in fp32 and
# |dot| < 2^24 so results are exact). Attention uses f32r Q/K for scores and
# bf16 for exp(scores)/V.
#
# Perf structure (vs naive): single batched DMA-transposes for activation
# layout, RoPE rotate-half via a PE permutation matmul (no partition-shift
# DMA pairs), softmax denominator fused into the attention-value matmul as
# an extra ones-column of V (output comes out [token, feat] so the BitNet
# re-quant absmax is a free-axis reduce instead of a partition tree), and
# weights streamed in large batched DMAs ordered so gathers and weight loads
# hide under projection matmuls.
import math

import numpy as np

import concourse.bacc as bacc
import concourse.bass as bass
import concourse.bass_isa as bass_isa
import concourse.mybir as mybir
import concourse.tile as tile

DT = mybir.dt
AF = mybir.ActivationFunctionType
ALU = mybir.AluOpType
AX = mybir.AxisListType

H = 4096
QH, KVH, D = 32, 8, 128     # query heads, kv heads, head dim
HB = H // 128               # 32 hidden blocks
NREP = QH // KVH
ROUND_C = 12582912.0        # 1.5 * 2**23: fp32 add forces round-to-nearest-even int
LN_EPS = 1e-5
QB = 127.0
SM_SCALE = 1.0 / math.sqrt(128.0)
VW = 132                    # padded vgo row width (128 feat + 1 ones + pad)


def build(n_cores=8, s_pc=256, stub_collectives=False, body_reps=1,
          skip_gb=False, dbg=False, trunc=0):
    """Build the SPMD Bass program (identical on all cores; per-core data via inputs)."""
    NT = s_pc // 128            # token tiles per core
    S = s_pc * n_cores
    KT = S // 128               # key-token tiles after gather
    f32, bf16, f32r, f8 = DT.float32, DT.bfloat16, DT.float32r, DT.float8e4
    f16 = DT.float16

    nc = bacc.Bacc("TRN2", target_bir_lowering=False, debug=False, num_devices=n_cores)

    x_d = nc.dram_tensor("x", [s_pc, H], f32, kind="ExternalInput").ap()
    g_d = nc.dram_tensor("lng", [1, H], f32, kind="ExternalInput").ap()
    b_d = nc.dram_tensor("lnb", [1, H], f32, kind="ExternalInput").ap()
    cos_d = nc.dram_tensor("cos4", [D, 4, s_pc], f32, kind="ExternalInput").ap()
    sin_d = nc.dram_tensor("sin4", [D, 4, s_pc], f32, kind="ExternalInput").ap()
    pm_d = nc.dram_tensor("permM", [128, 128], f32, kind="ExternalInput").ap()
    wq_d = nc.dram_tensor("wqt", [128, QH, HB, 128], f8, kind="ExternalInput").ap()
    wk_d = nc.dram_tensor("wkt", [128, KVH, HB, 128], f8, kind="ExternalInput").ap()
    wv_d = nc.dram_tensor("wvt", [128, HB, KVH * D], f8, kind="ExternalInput").ap()
    wo_d = nc.dram_tensor("wot", [128, HB, HB, 128], f8, kind="ExternalInput").ap()
    sc_d = nc.dram_tensor("wscal", [1, 4], f32, kind="ExternalInput").ap()
    y_d = nc.dram_tensor("yT", [H, s_pc], f32, kind="ExternalOutput").ap()
    dbg_d = None
    if dbg:
        dbg_d = {
            "dxT": nc.dram_tensor("dxT", [128, HB, s_pc], bf16, kind="ExternalOutput").ap(),
            "dxq": nc.dram_tensor("dxq", [NT, 128, H], bf16, kind="ExternalOutput").ap(),
            "dxs": nc.dram_tensor("dxs", [NT, 128, H], f32, kind="ExternalOutput").ap(),
            "dscr": nc.dram_tensor("dscr", [NT, 128, H], f32, kind="ExternalOutput").ap(),
            "dstat": nc.dram_tensor("dstat", [NT, 128, 2], f32, kind="ExternalOutput").ap(),
            "dR": nc.dram_tensor("dR", [128, s_pc], f32, kind="ExternalOutput").ap(),
            "dkOut": nc.dram_tensor("dkOut", [128, KVH, s_pc], f32r, kind="ExternalOutput").ap(),
            "dvsrc": nc.dram_tensor("dvsrc", [NT, 128, KVH * D], f16, kind="ExternalOutput").ap(),
            "dqT": nc.dram_tensor("dqT", [128, QH, s_pc], f32r, kind="ExternalOutput").ap(),
            "daoSt": nc.dram_tensor("daoSt", [128, QH, 2, 128], f32, kind="ExternalOutput").ap(),
            "dao2T": nc.dram_tensor("dao2T", [128, 2 * QH, 128], bf16, kind="ExternalOutput").ap(),
        }

    with tile.TileContext(nc) as tc:
        for rep in range(body_reps):
            _body(nc, tc, n_cores, s_pc, NT, KT,
                  x_d, g_d, b_d, cos_d, sin_d, pm_d, wq_d, wk_d, wv_d, wo_d,
                  sc_d, y_d, stub_collectives, pfx=f"r{rep}_",
                  skip_gb=skip_gb, dbg_d=dbg_d, trunc=trunc)
    nc.compile()
    return nc


def _body(nc, tc, n_cores, s_pc, NT, KT,
          x_d, g_d, b_d, cos_d, sin_d, pm_d, wq_d, wk_d, wv_d, wo_d, sc_d,
          y_d, stub_collectives=False, pfx="", skip_gb=False, dbg_d=None,
          trunc=0):
    f32, bf16, f32r, f8 = DT.float32, DT.bfloat16, DT.float32r, DT.float8e4
    f16 = DT.float16
    sync, vec, act, pe, gp = nc.sync, nc.vector, nc.scalar, nc.tensor, nc.gpsimd

    from contextlib import ExitStack

    def bcast_row(psb_pool, ones1, row, out_sb, n, name):
        """Replicate [1, n] row across 128 partitions via K=1 fp32 matmul
        (exact: 1.0 * a) then copy PSUM->SBUF."""
        for i, n0 in enumerate(range(0, n, 512)):
            nn = min(512, n - n0)
            ps = psb_pool.tile([128, 512], f32, name=f"{name}_ps{i}", tag="psb")
            pe.matmul(ps[:, 0:nn], ones1, row[:, n0:n0 + nn],
                      start=True, stop=True)
            vec.tensor_copy(out_sb[:, n0:n0 + nn], ps[:, 0:nn])

    def bcast_from_dram(psb_pool, row_pool, ones1, dram_row, out_sb, n, name):
        for i, n0 in enumerate(range(0, n, 512)):
            nn = min(512, n - n0)
            rt = row_pool.tile([1, 512], f32, name=f"{name}_row{i}", tag="brow")
            sync.dma_start(rt[:, 0:nn], dram_row[:, n0:n0 + nn])
            ps = psb_pool.tile([128, 512], f32, name=f"{name}_ps{i}", tag="psb")
            pe.matmul(ps[:, 0:nn], ones1, rt[:, 0:nn], start=True, stop=True)
            vec.tensor_copy(out_sb[:, n0:n0 + nn], ps[:, 0:nn])

    es = ExitStack()
    with es:
        # ---------------- long-lived pools ----------------
        constp = es.enter_context(tc.tile_pool(name=pfx + "constp", bufs=1))
        dramp = es.enter_context(tc.tile_pool(name=pfx + "dramp", bufs=1, space="DRAM"))

        cos4 = constp.tile([D, 4, s_pc], f32, name="cos4", tag="cos4")
        sin4 = constp.tile([D, 4, s_pc], f32, name="sin4", tag="sin4")
        # NOTE: a direct f32r DMA load > [128,1] corrupts concurrent DVE
        # tensor_scalar results on HW (observed empirically); load as f32 and
        # bit-copy into the f32r tile on DVE instead.
        PM = constp.tile([128, 128], f32r, name="PM", tag="PM")
        PMf = constp.tile([128, 128], f32, name="PMf", tag="PMf")
        if not (trunc & 16):
            sync.dma_start(cos4, cos_d)
            sync.dma_start(sin4, sin_d)
            if not (trunc & 64):
                sync.dma_start(PMf, pm_d)
                vec.tensor_copy(PM, PMf)
        ones1 = constp.tile([1, 128], f32, name="ones1", tag="ones1")
        vec.memset(ones1, 1.0)
        scal_sb = constp.tile([128, 4], f32, name="scal_sb", tag="scal_sb")
        scal_row = constp.tile([1, 4], f32, name="scal_row", tag="scal_row")
        sync.dma_start(scal_row, sc_d)
        sw_q, sw_k, sw_v, sw_o = (scal_sb[:, i:i + 1] for i in range(4))

        # per-token re-quant absmax accumulator (attention out), per q-half
        racc = constp.tile([128, 2], f32, name="racc", tag="racc")
        vec.memset(racc, 0.0)
        cpos = constp.tile([128, 1], f32, name="cpos", tag="cpos")
        vec.memset(cpos, ROUND_C)
        cneg = constp.tile([128, 1], f32, name="cneg", tag="cneg")
        vec.memset(cneg, -ROUND_C)
        ceps = constp.tile([128, 1], f32, name="ceps", tag="ceps")
        vec.memset(ceps, LN_EPS)

        # per-token dequant scale tiles (partition layout)
        r_tiles = []
        for t in range(NT):
            r_t = constp.tile([128, 1], f32, name=f"r_{t}", tag=f"r_{t}")
            r_tiles.append(r_t)

        r_dram = dramp.tile([1, s_pc], f32, name="r_dram", tag="r_dram")
        r2_dram = dramp.tile([1, s_pc], f32, name="r2_dram", tag="r2_dram")
        ksrc = dramp.tile([D, KVH, s_pc], f32r, name="ksrc", tag="ksrc")
        vsrc = dramp.tile([NT, 128, KVH * D], f16, name="vsrc", tag="vsrc")
        kv_space = "Local" if stub_collectives else "Shared"
        KG = dramp.tile([n_cores, D, KVH, s_pc], f32r, name="KG", tag="KG",
                        addr_space=kv_space)
        VG = dramp.tile([n_cores, NT, 128, KVH * D], f16, name="VG", tag="VG",
                        addr_space=kv_space)

        qTp = es.enter_context(tc.tile_pool(name=pfx + "qTp", bufs=1))

        with tc.tile_pool(name=pfx + "xTp", bufs=1) as xTp:
            # quantized+transposed activations [hid, tok] as bf16 integers
            xT = xTp.tile([128, HB, s_pc], bf16, name="xT", tag="xT")
            # per-token dequant scale replicated on all partitions
            R = xTp.tile([128, s_pc], f32, name="R", tag="R")

            # K and V weights are loaded upfront on the Act queue so they
            # stream during layernorm and never queue behind data-path DMAs.
            # The pool is closed right after phase 3 to free SBUF.
            es_kv = ExitStack()
            wkvp = es_kv.enter_context(tc.tile_pool(name=pfx + "wkvp", bufs=1))
            wk_sb = wkvp.tile([128, KVH, HB, 128], f8, name="wk", tag="wk")
            wv_sbs = []
            for kc in range(4):
                wv_sb = wkvp.tile([128, 8, KVH * D], f8, name=f"wv{kc}",
                                  tag=f"wv{kc}")
                wv_sbs.append(wv_sb)
            if not (trunc & 2):
                act.dma_start(wk_sb, wk_d)
                for kc in range(4):
                    act.dma_start(wv_sbs[kc], wv_d[:, kc * 8:(kc + 1) * 8, :])

            # ---------------- phase 1: layernorm + act quant ----------------
            with tc.tile_pool(name=pfx + "lnp", bufs=2) as lnp, \
                 tc.tile_pool(name=pfx + "scrp", bufs=(2 if trunc & 8 else 1)) as scrp, \
                 tc.tile_pool(name=pfx + "gbp", bufs=1) as gbp, \
                 tc.tile_pool(name=pfx + "statp", bufs=1) as statp, \
                 tc.tile_pool(name=pfx + "psb1", bufs=2, space="PSUM") as psb1, \
                 tc.tile_pool(name=pfx + "xqp", bufs=2) as xqp:
                if not skip_gb:
                    Gt = gbp.tile([128, H], f32, name="Gt", tag="Gt")
                    Bt = gbp.tile([128, H], f32, name="Bt", tag="Bt")
                    bcast_from_dram(psb1, gbp, ones1, g_d, Gt, H, "g")
                    bcast_from_dram(psb1, gbp, ones1, b_d, Bt, H, "b")
                if not (trunc & 16) and not (trunc & 32):
                    bcast_row(psb1, ones1, scal_row, scal_sb, 4, "sc")

                for t in range(NT):
                    xs = lnp.tile([128, H], f32, name=f"xs{t}", tag="xs")
                    xq = xqp.tile([128, H], bf16, name=f"xq{t}", tag="xq")
                    scr = scrp.tile([128, H], f32, name=f"scr{t}", tag="scr")
                    sync.dma_start(xs, x_d[t * 128:(t + 1) * 128, :])

                    # sum (DVE) and sum-of-squares (Act) run concurrently;
                    # var = E[x^2] - mu^2. Square's value output is scratch
                    # (only accum_out matters).
                    sumsq = statp.tile([128, 1], f32, name=f"sumsq{t}", tag=f"sumsq{t}")
                    act.activation(scr, xs, AF.Square, accum_out=sumsq)
                    nsum = statp.tile([128, 1], f32, name=f"nsum{t}", tag=f"nsum{t}")
                    vec.tensor_reduce(nsum, xs, axis=AX.X, op=ALU.add, negate=True)
                    nmu = statp.tile([128, 1], f32, name=f"nmu{t}", tag=f"nmu{t}")
                    vec.tensor_scalar_mul(nmu, nsum, 1.0 / H)
                    mu2 = statp.tile([128, 1], f32, name=f"mu2{t}", tag=f"mu2{t}")
                    vec.tensor_mul(mu2, nmu, nmu)
                    varv = statp.tile([128, 1], f32, name=f"varv{t}", tag=f"varv{t}")
                    vec.scalar_tensor_tensor(varv, sumsq, 1.0 / H, mu2,
                                             op0=ALU.mult, op1=ALU.subtract)
                    stdv = statp.tile([128, 1], f32, name=f"stdv{t}", tag=f"stdv{t}")
                    act.activation(stdv, varv, AF.Sqrt, bias=ceps)
                    rstd = statp.tile([128, 1], f32, name=f"rstd{t}", tag=f"rstd{t}")
                    vec.reciprocal(rstd, stdv)
                    nmr = statp.tile([128, 1], f32, name=f"nmr{t}", tag=f"nmr{t}")
                    vec.tensor_mul(nmr, nmu, rstd)
                    # normed = x*rstd + (-mu*rstd), then *g + b (in place)
                    act.activation(xs, xs, AF.Identity, bias=nmr, scale=rstd)
                    if not skip_gb:
                        vec.tensor_mul(xs, xs, Gt)
                        vec.tensor_add(xs, xs, Bt)

                    am = statp.tile([128, 1], f32, name=f"am{t}", tag=f"am{t}")
                    vec.tensor_reduce(am, xs, axis=AX.X, op=ALU.max,
                                      apply_absolute_value=True)
                    amc = statp.tile([128, 1], f32, name=f"amc{t}", tag=f"amc{t}")
                    vec.tensor_scalar_max(amc, am, 1e-5)
                    r_t = r_tiles[t]
                    vec.tensor_scalar_mul(r_t, amc, 1.0 / QB)
                    inv = statp.tile([128, 1], f32, name=f"inv{t}", tag=f"inv{t}")
                    vec.reciprocal(inv, amc)
                    scq = statp.tile([128, 1], f32, name=f"scq{t}", tag=f"scq{t}")
                    vec.tensor_scalar_mul(scq, inv, QB)

                    # n = round(normed * scq), exact via +C trick; write as bf16 ints
                    vec.tensor_scalar(scr, xs, scq, ROUND_C, ALU.mult, ALU.add)
                    if dbg_d is not None:
                        sync.dma_start(dbg_d["dscr"][t], scr)
                        sync.dma_start(dbg_d["dstat"][t][:, 0:1], scq)
                        sync.dma_start(dbg_d["dstat"][t][:, 1:2], amc)
                    vec.tensor_scalar_add(xq, scr, -ROUND_C)

                    # one batched xbar transpose into [hid, tok] layout
                    if trunc & 4:
                        for h in range(HB):
                            sync.dma_start(xT[:, h, t * 128:(t + 1) * 128],
                                           xq[:, h * 128:(h + 1) * 128],
                                           transpose=True)
                    else:
                        sync.dma_start(xT[:, :, t * 128:(t + 1) * 128], xq,
                                       transpose=True)
                    if dbg_d is not None:
                        sync.dma_start(dbg_d["dxq"][t], xq)
                        sync.dma_start(dbg_d["dxs"][t], xs)
                    # export per-token scale
                    sync.dma_start(r_dram[0, t * 128:(t + 1) * 128], r_t[:, 0])

                r_row = constp.tile([1, s_pc], f32, name="r_row", tag="r_row")
                sync.dma_start(r_row, r_dram[:])
                bcast_row(psb1, ones1, r_row, R, s_pc, "r")

            if trunc & 1:
                es_kv.close()
                return

            # ---------------- phase 2: V projection + gather ----------------
            # All wv chunks stay resident; token-tile 0's accumulation chain
            # runs first so it overlaps token-tile 1's layernorm.
            with tc.tile_pool(name=pfx + "psv", bufs=1, space="PSUM") as psvp, \
                 tc.tile_pool(name=pfx + "vdrp", bufs=2) as vdrp:
                psvs = [psvp.tile([128, KVH * D], f32, name=f"psv{t}",
                                  tag=f"psv{t}") for t in range(NT)]
                for t in range(NT):
                    for k in range(HB):
                        for n0 in (0, 512):
                            pe.matmul(psvs[t][:, n0:n0 + 512],
                                      xT[:, k, t * 128:(t + 1) * 128],
                                      wv_sbs[k // 8][:, k % 8, n0:n0 + 512],
                                      start=(k == 0), stop=(k == HB - 1))
                    vdr = vdrp.tile([128, KVH * D], f16, name=f"vdr{t}", tag="vdr")
                    vec.tensor_scalar(vdr, psvs[t], r_tiles[t], sw_v,
                                      ALU.mult, ALU.mult)
                    sync.dma_start(vsrc[t], vdr)
                    if dbg_d is not None:
                        sync.dma_start(dbg_d["dvsrc"][t], vdr)

            if stub_collectives:
                for r in range(n_cores):
                    sync.dma_start(VG[r], vsrc)
            else:
                gp.collective_compute(
                    "AllGather", ALU.bypass,
                    replica_groups=[list(range(n_cores))],
                    ins=[vsrc.opt()], outs=[VG.opt()])

            # ---------------- phase 3: K projection + rope + gather ----------
            with tc.tile_pool(name=pfx + "psk", bufs=2, space="PSUM") as pskp, \
                 tc.tile_pool(name=pfx + "psr", bufs=2, space="PSUM") as psrp, \
                 tc.tile_pool(name=pfx + "kdp", bufs=1) as kdp, \
                 tc.tile_pool(name=pfx + "ktp", bufs=2) as ktp:
                kdrA = kdp.tile([128, KVH, s_pc], f32r, name="kdrA", tag="kdrA")
                kOut = kdp.tile([128, KVH, s_pc], f32r, name="kOut", tag="kOut")
                for f in range(KVH):
                    ps = pskp.tile([128, s_pc], f32, name=f"psk{f}", tag="psk")
                    for k in range(HB):
                        pe.matmul(ps, wk_sb[:, f, k, :], xT[:, k, :],
                                  start=(k == 0), stop=(k == HB - 1))
                    vec.scalar_tensor_tensor(kdrA[:, f, :], ps, sw_k, R,
                                             op0=ALU.mult, op1=ALU.mult)
                for c in range(KVH // 4):
                    sl = kdrA[:, 4 * c:4 * c + 4, :]
                    rot = psrp.tile([128, 4 * s_pc], f32, name=f"krot{c}",
                                    tag="krot")
                    slf = sl.rearrange("p a b -> p (a b)")
                    for n0 in (0, 2 * s_pc):
                        pe.matmul(rot[:, n0:n0 + 2 * s_pc], PM,
                                  slf[:, n0:n0 + 2 * s_pc],
                                  start=True, stop=True)
                    kcs = ktp.tile([128, 4, s_pc], f32, name=f"kcs{c}", tag="kcs")
                    vec.tensor_mul(kcs, sl, cos4)
                    ksn = ktp.tile([128, 4, s_pc], f32, name=f"ksn{c}", tag="ksn")
                    vec.tensor_mul(ksn.rearrange("p a b -> p (a b)"), rot,
                                   sin4.rearrange("p a b -> p (a b)"))
                    vec.tensor_add(kOut[:, 4 * c:4 * c + 4, :], kcs, ksn)
                sync.dma_start(ksrc.rearrange("d f t -> d (f t)"),
                               kOut.rearrange("p a b -> p (a b)"))
                if dbg_d is not None:
                    sync.dma_start(dbg_d["dkOut"], kOut)

            if stub_collectives:
                for r in range(n_cores):
                    sync.dma_start(KG[r], ksrc)
            else:
                gp.collective_compute(
                    "AllGather", ALU.bypass,
                    replica_groups=[list(range(n_cores))],
                    ins=[ksrc.opt()], outs=[KG.opt()])

            es_kv.close()   # free K/V weight SBUF

            # ---------------- phase 4: Q projection + rope ----------------
            qTall = qTp.tile([128, QH, s_pc], f32r, name="qTall", tag="qTall")
            with tc.tile_pool(name=pfx + "wqp", bufs=2) as wqp, \
                 tc.tile_pool(name=pfx + "psq", bufs=4, space="PSUM") as psqp, \
                 tc.tile_pool(name=pfx + "psr2", bufs=2, space="PSUM") as psr2, \
                 tc.tile_pool(name=pfx + "qdp", bufs=2) as qdp, \
                 tc.tile_pool(name=pfx + "qtp", bufs=2) as qtp:
                for fc in range(QH // 4):
                    wq_sb = wqp.tile([128, 4, HB, 128], f8, name=f"wq{fc}",
                                     tag="wq")
                    act.dma_start(wq_sb, wq_d[:, 4 * fc:4 * fc + 4, :, :])
                    qdr4 = qdp.tile([128, 4, s_pc], f32r, name=f"qdr{fc}",
                                    tag="qdr")
                    for j in range(4):
                        f = fc * 4 + j
                        ps = psqp.tile([128, s_pc], f32, name=f"psq{f}", tag="psq")
                        for k in range(HB):
                            pe.matmul(ps, wq_sb[:, j, k, :], xT[:, k, :],
                                      start=(k == 0), stop=(k == HB - 1))
                        vec.scalar_tensor_tensor(qdr4[:, j, :], ps, sw_q, R,
                                                 op0=ALU.mult, op1=ALU.mult)
                    rot = psr2.tile([128, 4 * s_pc], f32, name=f"qrot{fc}",
                                    tag="qrot")
                    qdf = qdr4.rearrange("p a b -> p (a b)")
                    for n0 in (0, 2 * s_pc):
                        pe.matmul(rot[:, n0:n0 + 2 * s_pc], PM,
                                  qdf[:, n0:n0 + 2 * s_pc],
                                  start=True, stop=True)
                    qcs = qtp.tile([128, 4, s_pc], f32, name=f"qcs{fc}", tag="qcs")
                    vec.tensor_mul(qcs, qdr4, cos4)
                    qsn = qtp.tile([128, 4, s_pc], f32, name=f"qsn{fc}", tag="qsn")
                    vec.tensor_mul(qsn.rearrange("p a b -> p (a b)"), rot,
                                   sin4.rearrange("p a b -> p (a b)"))
                    vec.tensor_add(qTall[:, 4 * fc:4 * fc + 4, :], qcs, qsn)
            if dbg_d is not None:
                sync.dma_start(dbg_d["dxT"], xT)
                sync.dma_start(dbg_d["dR"], R)
                sync.dma_start(dbg_d["dqT"], qTall)

        # ---------------- phase 5: attention ----------------
        # attention output, normalized, [q-token(part), head, q-half, feat]
        aop = es.enter_context(tc.tile_pool(name=pfx + "aop", bufs=1))
        aoSt = aop.tile([128, QH, 2, 128], f32, name="aoSt", tag="aoSt")
        # O-projection weights double-buffer on the Act queue; the first two
        # batches are issued before attention so they stream underneath it.
        wop = es.enter_context(tc.tile_pool(name=pfx + "wop", bufs=2))
        wo_sbs = {}

        def load_wo(fc):
            wo_sb = wop.tile([128, 4, HB, 128], f8, name=f"wo{fc}", tag="wo")
            act.dma_start(wo_sb, wo_d[:, 4 * fc:4 * fc + 4, :, :])
            wo_sbs[fc] = wo_sb

        load_wo(0)
        load_wo(1)

        with tc.tile_pool(name=pfx + "kgp", bufs=2) as kgp, \
             tc.tile_pool(name=pfx + "vgp", bufs=2) as vgp, \
             tc.tile_pool(name=pfx + "ep", bufs=2) as ep, \
             tc.tile_pool(name=pfx + "pss", bufs=2, space="PSUM") as pssp, \
             tc.tile_pool(name=pfx + "av0", bufs=2, space="PSUM") as avp0, \
             tc.tile_pool(name=pfx + "av1", bufs=2, space="PSUM") as avp1, \
             tc.tile_pool(name=pfx + "dnp", bufs=2) as dnp:
            for g in range(KVH):
                kg_sb = kgp.tile([128, n_cores, s_pc], f32r, name=f"kg{g}",
                                 tag="kg")
                sync.dma_start(kg_sb, KG[:, :, g, :].rearrange("r d t -> d r t"))
                vgo = vgp.tile([128, KT, VW], f16, name=f"vg{g}", tag="vg")
                sync.dma_start(
                    vgo[:, :, 0:128],
                    VG.rearrange("r t p f -> p (r t) f")[:, :,
                                                         g * 128:(g + 1) * 128])
                vec.memset(vgo[:, :, 128:129], 1.0)
                for j in range(NREP):
                    hq = g * NREP + j
                    qh_ap = qTall[:, hq, :]
                    eT = ep.tile([128, KT, s_pc], f16, name=f"eT{g}_{j}",
                                 tag="eT")
                    GRP = min(4, KT)
                    for kt0 in range(0, KT, GRP):
                        pss = pssp.tile([128, GRP * s_pc], f32,
                                        name=f"pss{g}_{j}_{kt0}", tag="pss")
                        for u in range(GRP):
                            kt = kt0 + u
                            pe.matmul(pss[:, u * s_pc:(u + 1) * s_pc],
                                      kg_sb[:, kt // NT,
                                            (kt % NT) * 128:
                                            (kt % NT) * 128 + 128],
                                      qh_ap, start=True, stop=True)
                        act.activation(
                            eT[:, kt0:kt0 + GRP, :].rearrange("p a b -> p (a b)"),
                            pss, AF.Exp, scale=SM_SCALE)
                    # attention output + softmax denominator in one pass:
                    # out[q, 0:128] = sum_k e[k,q] v[k,:], out[q, 128] = sum_k e[k,q]
                    avs = [avp0.tile([128, VW], f32, name=f"av0_{g}_{j}", tag="av0"),
                           avp1.tile([128, VW], f32, name=f"av1_{g}_{j}", tag="av1")]
                    for kt in range(KT):
                        for qh in range(2):
                            pe.matmul(avs[qh][:, 0:129],
                                      eT[:, kt, qh * 128:(qh + 1) * 128],
                                      vgo[:, kt, 0:129],
                                      start=(kt == 0), stop=(kt == KT - 1))
                    for qh in range(2):
                        dn = dnp.tile([128, 1], f32, name=f"dn{g}_{j}_{qh}",
                                      tag="dn")
                        vec.reciprocal(dn, avs[qh][:, 128:129])
                        vec.tensor_scalar_mul(aoSt[:, hq, qh, :],
                                              avs[qh][:, 0:128], dn)
                        tred = dnp.tile([128, 1], f32, name=f"tr{g}_{j}_{qh}",
                                        tag="tred")
                        vec.tensor_reduce(tred, aoSt[:, hq, qh, :], axis=AX.X,
                                          op=ALU.max, apply_absolute_value=True)
                        vec.tensor_tensor(racc[:, qh:qh + 1], racc[:, qh:qh + 1],
                                          tred, ALU.max)

        # ---------------- phase 6: re-quant + O projection ----------------
        with tc.tile_pool(name=pfx + "q2p", bufs=1) as q2p, \
             tc.tile_pool(name=pfx + "t1p", bufs=4) as t1p, \
             tc.tile_pool(name=pfx + "wop", bufs=2) as wop, \
             tc.tile_pool(name=pfx + "psy", bufs=4, space="PSUM") as psyp, \
             tc.tile_pool(name=pfx + "psb5", bufs=1, space="PSUM") as psb5, \
             tc.tile_pool(name=pfx + "yp", bufs=2) as yp:
            amc2 = q2p.tile([128, 2], f32, name="amc2", tag="amc2")
            vec.tensor_scalar_max(amc2, racc, 1e-5)
            r2 = q2p.tile([128, 2], f32, name="r2", tag="r2")
            vec.tensor_scalar_mul(r2, amc2, 1.0 / QB)
            inv2 = q2p.tile([128, 2], f32, name="inv2", tag="inv2")
            vec.reciprocal(inv2, amc2)
            sc2 = q2p.tile([128, 2], f32, name="sc2", tag="sc2")
            vec.tensor_scalar_mul(sc2, inv2, QB)
            for qh in range(2):
                sync.dma_start(r2_dram[0, qh * 128:(qh + 1) * 128], r2[:, qh])
            r2_row = q2p.tile([1, s_pc], f32, name="r2_row", tag="r2_row")
            sync.dma_start(r2_row, r2_dram[:])
            R2 = q2p.tile([128, s_pc], f32, name="R2", tag="R2")
            bcast_row(psb5, ones1, r2_row, R2, s_pc, "r2")

            # re-quant: ao2 = round(aoSt * sc2) as bf16 ints, [q, head, qh, f]
            ao2 = q2p.tile([128, QH, 2, 128], bf16, name="ao2", tag="ao2")
            for qh in range(2):
                for c in range(4):
                    sl = aoSt[:, c * 8:(c + 1) * 8, qh, :]
                    t1 = t1p.tile([128, 8, 128], f32, name=f"t1_{qh}_{c}",
                                  tag="t1")
                    eng = vec if (c % 2 == 0) else None
                    if eng is not None:
                        vec.tensor_scalar(t1, sl, sc2[:, qh:qh + 1], ROUND_C,
                                          ALU.mult, ALU.add)
                        vec.tensor_scalar_add(ao2[:, c * 8:(c + 1) * 8, qh, :],
                                              t1, -ROUND_C)
                    else:
                        act.activation(t1, sl, AF.Identity,
                                       bias=cpos, scale=sc2[:, qh:qh + 1])
                        act.activation(ao2[:, c * 8:(c + 1) * 8, qh, :], t1,
                                       AF.Identity, bias=cneg, scale=1.0)

            # one batched xbar transpose: ao2T[f, (head,qh), q] holds the
            # O-projection moving operand [hid, tok]
            ao2T = q2p.tile([128, 2 * QH, 128], bf16, name="ao2T", tag="ao2T")
            sync.dma_start(ao2T, ao2.rearrange("p a b c -> p (a b c)"),
                           transpose=True)
            if dbg_d is not None:
                sync.dma_start(dbg_d["daoSt"], aoSt)
                sync.dma_start(dbg_d["dao2T"], ao2T)

            for fc in range(HB // 4):
                if fc >= 2:
                    load_wo(fc)
                wo_sb = wo_sbs[fc]
                yT4 = yp.tile([128, 4, s_pc], f32, name=f"yT{fc}", tag="yT")
                for j in range(4):
                    f = fc * 4 + j
                    ps = psyp.tile([128, s_pc], f32, name=f"psy{f}", tag="psy")
                    for k in range(HB):
                        pe.matmul(ps, wo_sb[:, j, k, :],
                                  ao2T[:, 2 * k:2 * k + 2, :].rearrange(
                                      "p a b -> p (a b)"),
                                  start=(k == 0), stop=(k == HB - 1))
                    vec.scalar_tensor_tensor(yT4[:, j, :], ps, sw_o, R2,
                                             op0=ALU.mult, op1=ALU.mult)
                sync.dma_start(
                    y_d.rearrange("(a p) t -> p a t", p=128)[:, 4 * fc:4 * fc + 4, :],
                    yT4)


# ---------------------------------------------------------------------------
# host side
# ---------------------------------------------------------------------------

def _weight_quant_host(W):
    """Mimic reference _weight_quant: returns ternary m in {-1,0,1} and the
    effective dequant scale (1/scale) as fp32."""
    W = np.asarray(W, dtype=np.float32)
    mean_abs = np.float32(np.mean(np.abs(W), dtype=np.float64))
    clipped = np.maximum(mean_abs, np.float32(1e-5))
    scale = np.float32(1.0) / clipped
    m = np.clip(np.round(W * scale), -1.0, 1.0).astype(np.float32)
    sw = np.float32(1.0) / scale    # dequant scale applied after int matmul
    return m, float(sw)


def _prep_weights(Wq, Wk, Wv, Wo):
    mq, swq = _weight_quant_host(Wq)
    mk, swk = _weight_quant_host(Wk)
    mv, swv = _weight_quant_host(Wv)
    mo, swo = _weight_quant_host(Wo)

    def blocked(mT, fb):  # mT: [H, out] -> [128, fb, HB, 128]
        return np.ascontiguousarray(
            mT.reshape(HB, 128, fb, 128).transpose(1, 2, 0, 3)
        ).astype(np.float32)

    wqt = blocked(mq.T, QH)
    wkt = blocked(mk.T, KVH)
    wot = blocked(mo.T, HB)
    wvt = np.ascontiguousarray(
        mv.T.reshape(HB, 128, KVH * D).transpose(1, 0, 2)).astype(np.float32)
    f8np = mybir.dt.np(mybir.dt.float8e4)
    tof8 = lambda a: a.astype(f8np)
    return (tof8(wqt), tof8(wkt), tof8(wvt), tof8(wot),
            np.array([[swq, swk, swv, swo]], dtype=np.float32))


def _rope_tables(S):
    inv = (1.0 / (10000.0 ** (np.arange(0, D, 2, dtype=np.float32)
                              / np.float32(D)))).astype(np.float32)
    pos = np.arange(S, dtype=np.float32)
    fr = pos[:, None] * inv[None, :]          # [S, 64]
    emb = np.concatenate([fr, fr], axis=1)    # [S, D]
    cosT = np.cos(emb).T.astype(np.float32).copy()   # [D, S]
    sinT = np.sin(emb).T.astype(np.float32).copy()
    sinT[0:64, :] *= -1.0                      # sign baked for rotate-half
    return cosT, sinT


def _perm_matrix():
    # rot[i, t] = kdr[(i+64) % 128, t] as out = P^T @ kdr with stationary P:
    # P[c, i] = 1 iff c == (i+64) % 128
    P = np.zeros((128, 128), dtype=np.float32)
    for i in range(128):
        P[(i + 64) % 128, i] = 1.0
    return P


def _in_maps(inputs, n_cores=8, s_pc=256):
    hs = np.asarray(inputs["hidden_states"], dtype=np.float32)
    ln_w = np.asarray(inputs["ln_w"], dtype=np.float32).reshape(1, H)
    ln_b = np.asarray(inputs["ln_b"], dtype=np.float32).reshape(1, H)
    wqt, wkt, wvt, wot, wscal = _prep_weights(
        inputs["Wq"], inputs["Wk"], inputs["Wv"], inputs["Wo"])
    S = hs.shape[1]
    cosT, sinT = _rope_tables(S)
    P = _perm_matrix()
    maps = []
    for c in range(n_cores):
        sl = slice(c * s_pc, (c + 1) * s_pc)
        cos4 = np.ascontiguousarray(
            np.broadcast_to(cosT[:, None, sl], (D, 4, s_pc)))
        sin4 = np.ascontiguousarray(
            np.broadcast_to(sinT[:, None, sl], (D, 4, s_pc)))
        maps.append({
            "x": np.ascontiguousarray(hs[0, sl, :]),
            "lng": ln_w, "lnb": ln_b,
            "cos4": cos4, "sin4": sin4, "permM": P,
            "wqt": wqt, "wkt": wkt, "wvt": wvt, "wot": wot,
            "wscal": wscal,
        })
    return maps


_CACHED = {}


def _run(inputs, trace=False, n_cores=8, s_pc=256, return_res=False):
    from concourse.bass_utils import run_bass_kernel_spmd
    skip_gb = bool(
        np.allclose(np.asarray(inputs["ln_w"]), 1.0)
        and np.allclose(np.asarray(inputs["ln_b"]), 0.0))
    key = (n_cores, s_pc, skip_gb)
    if key not in _CACHED:
        _CACHED[key] = build(n_cores, s_pc, skip_gb=skip_gb)
    nc = _CACHED[key]
    maps = _in_maps(inputs, n_cores, s_pc)
    res = run_bass_kernel_spmd(nc, maps, list(range(n_cores)), trace=trace)
    parts = [res.results[c]["yT"].T for c in range(n_cores)]
    y = np.concatenate(parts, axis=0)[None, :, :].astype(np.float32)
    if return_res:
        return y, res.exec_time_ns, res
    return y, res.exec_time_ns


def kernel(**inputs):
    y, _ = _run(inputs, trace=False)
    return y


# revision 59
# speedup vs baseline: 14.5078x; 1.3926x over previous
# BitNet GQA attention block on 8 Trainium2 NeuronCores.
#
# Sharding: data parallel over sequence (256 tokens/core). K/V are computed
# per-core for the local tokens, RoPE'd, then AllGathered so every core can
# run full (non-causal) attention for its own query tokens.
#
# Numerics: projections run as integer-exact matmuls -- 8-bit quantized
# activations are integers <=127 held in bf16, ternary weights are -1/0/1
# held in fp8e4m3 (both exact; mixed-dtype matmul accumulates in fp32 and
# |dot| < 2^24 so results are exact). Attention uses f32r Q/K for scores and
# bf16 for exp(scores)/V.
#
# Perf structure (vs naive): single batched DMA-transposes for activation
# layout, RoPE rotate-half via a PE permutation matmul (no partition-shift
# DMA pairs), softmax denominator fused into the attention-value matmul as
# an extra ones-column of V (output comes out [token, feat] so the BitNet
# re-quant absmax is a free-axis reduce instead of a partition tree), and
# weights streamed in large batched DMAs ordered so gathers and weight loads
# hide under projection matmuls.
import math

import numpy as np

import concourse.bacc as bacc
import concourse.bass as bass
import concourse.bass_isa as bass_isa
import concourse.mybir as mybir
import concourse.tile as tile

DT = mybir.dt
AF = mybir.ActivationFunctionType
ALU = mybir.AluOpType
AX = mybir.AxisListType

H = 4096
QH, KVH, D = 32, 8, 128     # query heads, kv heads, head dim
HB = H // 128               # 32 hidden blocks
NREP = QH // KVH
ROUND_C = 12582912.0        # 1.5 * 2**23: fp32 add forces round-to-nearest-even int
LN_EPS = 1e-5
QB = 127.0
SM_SCALE = 1.0 / math.sqrt(128.0)
VW = 132                    # padded vgo row width (128 feat + 1 ones + pad)


def build(n_cores=8, s_pc=256, stub_collectives=False, body_reps=1,
          skip_gb=False, dbg=False, trunc=0):
    """Build the SPMD Bass program (identical on all cores; per-core data via inputs)."""
    NT = s_pc // 128            # token tiles per core
    S = s_pc * n_cores
    KT = S // 128               # key-token tiles after gather
    f32, bf16, f32r, f8 = DT.float32, DT.bfloat16, DT.float32r, DT.float8e4
    f16 = DT.float16

    nc = bacc.Bacc("TRN2", target_bir_lowering=False, debug=False, num_devices=n_cores)

    x_d = nc.dram_tensor("x", [s_pc, H], f32, kind="ExternalInput").ap()
    g_d = nc.dram_tensor("lng", [1, H], f32, kind="ExternalInput").ap()
    b_d = nc.dram_tensor("lnb", [1, H], f32, kind="ExternalInput").ap()
    cos_d = nc.dram_tensor("cos4", [D, 4, s_pc], f32, kind="ExternalInput").ap()
    sin_d = nc.dram_tensor("sin4", [D, 4, s_pc], f32, kind="ExternalInput").ap()
    pm_d = nc.dram_tensor("permM", [128, 128], f32, kind="ExternalInput").ap()
    wq_d = nc.dram_tensor("wqt", [128, QH, HB, 128], f8, kind="ExternalInput").ap()
    wk_d = nc.dram_tensor("wkt", [128, KVH, HB, 128], f8, kind="ExternalInput").ap()
    wv_d = nc.dram_tensor("wvt", [128, HB, KVH * D], f8, kind="ExternalInput").ap()
    wo_d = nc.dram_tensor("wot", [128, HB, HB, 128], f8, kind="ExternalInput").ap()
    sc_d = nc.dram_tensor("wscal", [1, 4], f32, kind="ExternalInput").ap()
    y_d = nc.dram_tensor("yT", [H, s_pc], f32, kind="ExternalOutput").ap()
    dbg_d = None
    if dbg:
        dbg_d = {
            "dxT": nc.dram_tensor("dxT", [128, HB, s_pc], bf16, kind="ExternalOutput").ap(),
            "dxq": nc.dram_tensor("dxq", [NT, 128, H], bf16, kind="ExternalOutput").ap(),
            "dxs": nc.dram_tensor("dxs", [NT, 128, H], f32, kind="ExternalOutput").ap(),
            "dscr": nc.dram_tensor("dscr", [NT, 128, H], f32, kind="ExternalOutput").ap(),
            "dstat": nc.dram_tensor("dstat", [NT, 128, 2], f32, kind="ExternalOutput").ap(),
            "dR": nc.dram_tensor("dR", [128, s_pc], f32, kind="ExternalOutput").ap(),
            "dkOut": nc.dram_tensor("dkOut", [128, KVH, s_pc], f32r, kind="ExternalOutput").ap(),
            "dvsrc": nc.dram_tensor("dvsrc", [NT, 128, KVH * D], f16, kind="ExternalOutput").ap(),
            "dqT": nc.dram_tensor("dqT", [128, QH, s_pc], f32r, kind="ExternalOutput").ap(),
            "daoSt": nc.dram_tensor("daoSt", [128, QH, 2, 128], f32, kind="ExternalOutput").ap(),
            "dao2T": nc.dram_tensor("dao2T", [128, 2 * QH, 128], bf16, kind="ExternalOutput").ap(),
        }

    with tile.TileContext(nc) as tc:
        for rep in range(body_reps):
            _body(nc, tc, n_cores, s_pc, NT, KT,
                  x_d, g_d, b_d, cos_d, sin_d, pm_d, wq_d, wk_d, wv_d, wo_d,
                  sc_d, y_d, stub_collectives, pfx=f"r{rep}_",
                  skip_gb=skip_gb, dbg_d=dbg_d, trunc=trunc)
    nc.compile()
    return nc


def _body(nc, tc, n_cores, s_pc, NT, KT,
          x_d, g_d, b_d, cos_d, sin_d, pm_d, wq_d, wk_d, wv_d, wo_d, sc_d,
          y_d, stub_collectives=False, pfx="", skip_gb=False, dbg_d=None,
          trunc=0):
    f32, bf16, f32r, f8 = DT.float32, DT.bfloat16, DT.float32r, DT.float8e4
    f16 = DT.float16
    sync, vec, act, pe, gp = nc.sync, nc.vector, nc.scalar, nc.tensor, nc.gpsimd

    from contextlib import ExitStack

    def bcast_row(psb_pool, ones1, row, out_sb, n, name):
        """Replicate [1, n] row across 128 partitions via K=1 fp32 matmul
        (exact: 1.0 * a) then copy PSUM->SBUF."""
        for i, n0 in enumerate(range(0, n, 512)):
            nn = min(512, n - n0)
            ps = psb_pool.tile([128, 512], f32, name=f"{name}_ps{i}", tag="psb")
            pe.matmul(ps[:, 0:nn], ones1, row[:, n0:n0 + nn],
                      start=True, stop=True)
            vec.tensor_copy(out_sb[:, n0:n0 + nn], ps[:, 0:nn])

    def bcast_from_dram(psb_pool, row_pool, ones1, dram_row, out_sb, n, name):
        for i, n0 in enumerate(range(0, n, 512)):
            nn = min(512, n - n0)
            rt = row_pool.tile([1, 512], f32, name=f"{name}_row{i}", tag="brow")
            sync.dma_start(rt[:, 0:nn], dram_row[:, n0:n0 + nn])
            ps = psb_pool.tile([128, 512], f32, name=f"{name}_ps{i}", tag="psb")
            pe.matmul(ps[:, 0:nn], ones1, rt[:, 0:nn], start=True, stop=True)
            vec.tensor_copy(out_sb[:, n0:n0 + nn], ps[:, 0:nn])

    es = ExitStack()
    with es:
        # ---------------- long-lived pools ----------------
        constp = es.enter_context(tc.tile_pool(name=pfx + "constp", bufs=1))
        dramp = es.enter_context(tc.tile_pool(name=pfx + "dramp", bufs=1, space="DRAM"))

        cos4 = constp.tile([D, 4, s_pc], f32, name="cos4", tag="cos4")
        sin4 = constp.tile([D, 4, s_pc], f32, name="sin4", tag="sin4")
        # NOTE: a direct f32r DMA load > [128,1] corrupts concurrent DVE
        # tensor_scalar results on HW (observed empirically); load as f32 and
        # bit-copy into the f32r tile on DVE instead.
        PM = constp.tile([128, 128], f32r, name="PM", tag="PM")
        PMf = constp.tile([128, 128], f32, name="PMf", tag="PMf")
        if not (trunc & 16):
            sync.dma_start(cos4, cos_d)
            sync.dma_start(sin4, sin_d)
            if not (trunc & 64):
                sync.dma_start(PMf, pm_d)
                vec.tensor_copy(PM, PMf)
        ones1 = constp.tile([1, 128], f32, name="ones1", tag="ones1")
        vec.memset(ones1, 1.0)
        scal_sb = constp.tile([128, 4], f32, name="scal_sb", tag="scal_sb")
        scal_row = constp.tile([1, 4], f32, name="scal_row", tag="scal_row")
        sync.dma_start(scal_row, sc_d)
        sw_q, sw_k, sw_v, sw_o = (scal_sb[:, i:i + 1] for i in range(4))

        # per-token re-quant absmax accumulator (attention out), per q-half
        racc = constp.tile([128, 2], f32, name="racc", tag="racc")
        vec.memset(racc, 0.0)
        cpos = constp.tile([128, 1], f32, name="cpos", tag="cpos")
        vec.memset(cpos, ROUND_C)
        cneg = constp.tile([128, 1], f32, name="cneg", tag="cneg")
        vec.memset(cneg, -ROUND_C)
        ceps = constp.tile([128, 1], f32, name="ceps", tag="ceps")
        vec.memset(ceps, LN_EPS)

        # per-token dequant scale tiles (partition layout)
        r_tiles = []
        for t in range(NT):
            r_t = constp.tile([128, 1], f32, name=f"r_{t}", tag=f"r_{t}")
            r_tiles.append(r_t)

        r_dram = dramp.tile([1, s_pc], f32, name="r_dram", tag="r_dram")
        r2_dram = dramp.tile([1, s_pc], f32, name="r2_dram", tag="r2_dram")
        ksrc = dramp.tile([D, KVH, s_pc], f32r, name="ksrc", tag="ksrc")
        vsrc = dramp.tile([NT, 128, KVH * D], f16, name="vsrc", tag="vsrc")
        kv_space = "Local" if stub_collectives else "Shared"
        KG = dramp.tile([n_cores, D, KVH, s_pc], f32r, name="KG", tag="KG",
                        addr_space=kv_space)
        VG = dramp.tile([n_cores, NT, 128, KVH * D], f16, name="VG", tag="VG",
                        addr_space=kv_space)

        qTp = es.enter_context(tc.tile_pool(name=pfx + "qTp", bufs=1))

        with tc.tile_pool(name=pfx + "xTp", bufs=1) as xTp:
            # quantized+transposed activations [hid, tok] as bf16 integers
            xT = xTp.tile([128, HB, s_pc], bf16, name="xT", tag="xT")
            # per-token dequant scale replicated on all partitions
            R = xTp.tile([128, s_pc], f32, name="R", tag="R")

            # K and V weights are loaded upfront on the Act queue so they
            # stream during layernorm and never queue behind data-path DMAs.
            # The pool is closed right after phase 3 to free SBUF.
            es_kv = ExitStack()
            wkvp = es_kv.enter_context(tc.tile_pool(name=pfx + "wkvp", bufs=1))
            wk_sb = wkvp.tile([128, KVH, HB, 128], f8, name="wk", tag="wk")
            wv_sbs = []
            for kc in range(4):
                wv_sb = wkvp.tile([128, 8, KVH * D], f8, name=f"wv{kc}",
                                  tag=f"wv{kc}")
                wv_sbs.append(wv_sb)
            if not (trunc & 2):
                act.dma_start(wk_sb, wk_d)
                for kc in range(4):
                    act.dma_start(wv_sbs[kc], wv_d[:, kc * 8:(kc + 1) * 8, :])

            # ---------------- phase 1: layernorm + act quant ----------------
            with tc.tile_pool(name=pfx + "lnp", bufs=2) as lnp, \
                 tc.tile_pool(name=pfx + "scrp", bufs=(2 if trunc & 8 else 1)) as scrp, \
                 tc.tile_pool(name=pfx + "gbp", bufs=1) as gbp, \
                 tc.tile_pool(name=pfx + "statp", bufs=1) as statp, \
                 tc.tile_pool(name=pfx + "psb1", bufs=2, space="PSUM") as psb1, \
                 tc.tile_pool(name=pfx + "xqp", bufs=2) as xqp:
                if not skip_gb:
                    Gt = gbp.tile([128, H], f32, name="Gt", tag="Gt")
                    Bt = gbp.tile([128, H], f32, name="Bt", tag="Bt")
                    bcast_from_dram(psb1, gbp, ones1, g_d, Gt, H, "g")
                    bcast_from_dram(psb1, gbp, ones1, b_d, Bt, H, "b")
                if not (trunc & 16) and not (trunc & 32):
                    bcast_row(psb1, ones1, scal_row, scal_sb, 4, "sc")

                for t in range(NT):
                    xs = lnp.tile([128, H], f32, name=f"xs{t}", tag="xs")
                    xq = xqp.tile([128, H], bf16, name=f"xq{t}", tag="xq")
                    scr = scrp.tile([128, H], f32, name=f"scr{t}", tag="scr")
                    sync.dma_start(xs, x_d[t * 128:(t + 1) * 128, :])

                    # sum (DVE) and sum-of-squares (Act) run concurrently;
                    # var = E[x^2] - mu^2. Square's value output is scratch
                    # (only accum_out matters).
                    sumsq = statp.tile([128, 1], f32, name=f"sumsq{t}", tag=f"sumsq{t}")
                    act.activation(scr, xs, AF.Square, accum_out=sumsq)
                    nsum = statp.tile([128, 1], f32, name=f"nsum{t}", tag=f"nsum{t}")
                    vec.tensor_reduce(nsum, xs, axis=AX.X, op=ALU.add, negate=True)
                    nmu = statp.tile([128, 1], f32, name=f"nmu{t}", tag=f"nmu{t}")
                    vec.tensor_scalar_mul(nmu, nsum, 1.0 / H)
                    mu2 = statp.tile([128, 1], f32, name=f"mu2{t}", tag=f"mu2{t}")
                    vec.tensor_mul(mu2, nmu, nmu)
                    varv = statp.tile([128, 1], f32, name=f"varv{t}", tag=f"varv{t}")
                    vec.scalar_tensor_tensor(varv, sumsq, 1.0 / H, mu2,
                                             op0=ALU.mult, op1=ALU.subtract)
                    stdv = statp.tile([128, 1], f32, name=f"stdv{t}", tag=f"stdv{t}")
                    act.activation(stdv, varv, AF.Sqrt, bias=ceps)
                    rstd = statp.tile([128, 1], f32, name=f"rstd{t}", tag=f"rstd{t}")
                    vec.reciprocal(rstd, stdv)
                    nmr = statp.tile([128, 1], f32, name=f"nmr{t}", tag=f"nmr{t}")
                    vec.tensor_mul(nmr, nmu, rstd)
                    # normed = x*rstd + (-mu*rstd), then *g + b (in place)
                    act.activation(xs, xs, AF.Identity, bias=nmr, scale=rstd)
                    if not skip_gb:
                        vec.tensor_mul(xs, xs, Gt)
                        vec.tensor_add(xs, xs, Bt)

                    am = statp.tile([128, 1], f32, name=f"am{t}", tag=f"am{t}")
                    vec.tensor_reduce(am, xs, axis=AX.X, op=ALU.max,
                                      apply_absolute_value=True)
                    amc = statp.tile([128, 1], f32, name=f"amc{t}", tag=f"amc{t}")
                    vec.tensor_scalar_max(amc, am, 1e-5)
                    r_t = r_tiles[t]
                    vec.tensor_scalar_mul(r_t, amc, 1.0 / QB)
                    inv = statp.tile([128, 1], f32, name=f"inv{t}", tag=f"inv{t}")
                    vec.reciprocal(inv, amc)
                    scq = statp.tile([128, 1], f32, name=f"scq{t}", tag=f"scq{t}")
                    vec.tensor_scalar_mul(scq, inv, QB)

                    # n = round(normed * scq), exact via +C trick; write as bf16 ints
                    vec.tensor_scalar(scr, xs, scq, ROUND_C, ALU.mult, ALU.add)
                    if dbg_d is not None:
                        sync.dma_start(dbg_d["dscr"][t], scr)
                        sync.dma_start(dbg_d["dstat"][t][:, 0:1], scq)
                        sync.dma_start(dbg_d["dstat"][t][:, 1:2], amc)
                    vec.tensor_scalar_add(xq, scr, -ROUND_C)

                    # one batched xbar transpose into [hid, tok] layout
                    if trunc & 4:
                        for h in range(HB):
                            sync.dma_start(xT[:, h, t * 128:(t + 1) * 128],
                                           xq[:, h * 128:(h + 1) * 128],
                                           transpose=True)
                    else:
                        sync.dma_start(xT[:, :, t * 128:(t + 1) * 128], xq,
                                       transpose=True)
                    if dbg_d is not None:
                        sync.dma_start(dbg_d["dxq"][t], xq)
                        sync.dma_start(dbg_d["dxs"][t], xs)
                    # export per-token scale
                    sync.dma_start(r_dram[0, t * 128:(t + 1) * 128], r_t[:, 0])

                r_row = constp.tile([1, s_pc], f32, name="r_row", tag="r_row")
                sync.dma_start(r_row, r_dram[:])
                bcast_row(psb1, ones1, r_row, R, s_pc, "r")

            if trunc & 1:
                es_kv.close()
                return

            # ---------------- phase 2: V projection + gather ----------------
            # All wv chunks stay resident; token-tile 0's accumulation chain
            # runs first so it overlaps token-tile 1's layernorm.
            with tc.tile_pool(name=pfx + "psv", bufs=1, space="PSUM") as psvp, \
                 tc.tile_pool(name=pfx + "vdrp", bufs=2) as vdrp:
                psvs = [psvp.tile([128, KVH * D], f32, name=f"psv{t}",
                                  tag=f"psv{t}") for t in range(NT)]
                for t in range(NT):
                    for k in range(HB):
                        for n0 in (0, 512):
                            pe.matmul(psvs[t][:, n0:n0 + 512],
                                      xT[:, k, t * 128:(t + 1) * 128],
                                      wv_sbs[k // 8][:, k % 8, n0:n0 + 512],
                                      start=(k == 0), stop=(k == HB - 1))
                    vdr = vdrp.tile([128, KVH * D], f16, name=f"vdr{t}", tag="vdr")
                    vec.tensor_scalar(vdr, psvs[t], r_tiles[t], sw_v,
                                      ALU.mult, ALU.mult)
                    sync.dma_start(vsrc[t], vdr)
                    if dbg_d is not None:
                        sync.dma_start(dbg_d["dvsrc"][t], vdr)

            if stub_collectives:
                for r in range(n_cores):
                    sync.dma_start(VG[r], vsrc)
            else:
                gp.collective_compute(
                    "AllGather", ALU.bypass,
                    replica_groups=[list(range(n_cores))],
                    ins=[vsrc.opt()], outs=[VG.opt()])

            # ---------------- phase 3: K projection + rope + gather ----------
            with tc.tile_pool(name=pfx + "psk", bufs=2, space="PSUM") as pskp, \
                 tc.tile_pool(name=pfx + "psr", bufs=2, space="PSUM") as psrp, \
                 tc.tile_pool(name=pfx + "kdp", bufs=1) as kdp, \
                 tc.tile_pool(name=pfx + "ktp", bufs=2) as ktp:
                kdrA = kdp.tile([128, KVH, s_pc], f32r, name="kdrA", tag="kdrA")
                kOut = kdp.tile([128, KVH, s_pc], f32r, name="kOut", tag="kOut")
                for f in range(KVH):
                    ps = pskp.tile([128, s_pc], f32, name=f"psk{f}", tag="psk")
                    for k in range(HB):
                        pe.matmul(ps, wk_sb[:, f, k, :], xT[:, k, :],
                                  start=(k == 0), stop=(k == HB - 1))
                    vec.scalar_tensor_tensor(kdrA[:, f, :], ps, sw_k, R,
                                             op0=ALU.mult, op1=ALU.mult)
                for c in range(KVH // 4):
                    sl = kdrA[:, 4 * c:4 * c + 4, :]
                    rot = psrp.tile([128, 4 * s_pc], f32, name=f"krot{c}",
                                    tag="krot")
                    slf = sl.rearrange("p a b -> p (a b)")
                    for n0 in (0, 2 * s_pc):
                        pe.matmul(rot[:, n0:n0 + 2 * s_pc], PM,
                                  slf[:, n0:n0 + 2 * s_pc],
                                  start=True, stop=True)
                    kcs = ktp.tile([128, 4, s_pc], f32, name=f"kcs{c}", tag="kcs")
                    vec.tensor_mul(kcs, sl, cos4)
                    ksn = ktp.tile([128, 4, s_pc], f32, name=f"ksn{c}", tag="ksn")
                    vec.tensor_mul(ksn.rearrange("p a b -> p (a b)"), rot,
                                   sin4.rearrange("p a b -> p (a b)"))
                    vec.tensor_add(kOut[:, 4 * c:4 * c + 4, :], kcs, ksn)
                sync.dma_start(ksrc.rearrange("d f t -> d (f t)"),
                               kOut.rearrange("p a b -> p (a b)"))
                if dbg_d is not None:
                    sync.dma_start(dbg_d["dkOut"], kOut)

            if stub_collectives:
                for r in range(n_cores):
                    sync.dma_start(KG[r], ksrc)
            else:
                gp.collective_compute(
                    "AllGather", ALU.bypass,
                    replica_groups=[list(range(n_cores))],
                    ins=[ksrc.opt()], outs=[KG.opt()])

            es_kv.close()   # free K/V weight SBUF

            # ---------------- phase 4: Q projection + rope ----------------
            qTall = qTp.tile([128, QH, s_pc], f32r, name="qTall", tag="qTall")
            with tc.tile_pool(name=pfx + "wqp", bufs=2) as wqp, \
                 tc.tile_pool(name=pfx + "psq", bufs=4, space="PSUM") as psqp, \
                 tc.tile_pool(name=pfx + "psr2", bufs=2, space="PSUM") as psr2, \
                 tc.tile_pool(name=pfx + "qdp", bufs=2) as qdp, \
                 tc.tile_pool(name=pfx + "qtp", bufs=2) as qtp:
                for fc in range(QH // 4):
                    wq_sb = wqp.tile([128, 4, HB, 128], f8, name=f"wq{fc}",
                                     tag="wq")
                    act.dma_start(wq_sb, wq_d[:, 4 * fc:4 * fc + 4, :, :])
                    qdr4 = qdp.tile([128, 4, s_pc], f32r, name=f"qdr{fc}",
                                    tag="qdr")
                    for j in range(4):
                        f = fc * 4 + j
                        ps = psqp.tile([128, s_pc], f32, name=f"psq{f}", tag="psq")
                        for k in range(HB):
                            pe.matmul(ps, wq_sb[:, j, k, :], xT[:, k, :],
                                      start=(k == 0), stop=(k == HB - 1))
                        vec.scalar_tensor_tensor(qdr4[:, j, :], ps, sw_q, R,
                                                 op0=ALU.mult, op1=ALU.mult)
                    rot = psr2.tile([128, 4 * s_pc], f32, name=f"qrot{fc}",
                                    tag="qrot")
                    qdf = qdr4.rearrange("p a b -> p (a b)")
                    for n0 in (0, 2 * s_pc):
                        pe.matmul(rot[:, n0:n0 + 2 * s_pc], PM,
                                  qdf[:, n0:n0 + 2 * s_pc],
                                  start=True, stop=True)
                    qcs = qtp.tile([128, 4, s_pc], f32, name=f"qcs{fc}", tag="qcs")
                    vec.tensor_mul(qcs, qdr4, cos4)
                    qsn = qtp.tile([128, 4, s_pc], f32, name=f"qsn{fc}", tag="qsn")
                    vec.tensor_mul(qsn.rearrange("p a b -> p (a b)"), rot,
                                   sin4.rearrange("p a b -> p (a b)"))
                    vec.tensor_add(qTall[:, 4 * fc:4 * fc + 4, :], qcs, qsn)
            if dbg_d is not None:
                sync.dma_start(dbg_d["dxT"], xT)
                sync.dma_start(dbg_d["dR"], R)
                sync.dma_start(dbg_d["dqT"], qTall)

        # ---------------- phase 5: attention ----------------
        # attention output, normalized, [q-token(part), head, q-half, feat]
        aop = es.enter_context(tc.tile_pool(name=pfx + "aop", bufs=1))
        aoSt = aop.tile([128, QH, 2, 128], f32, name="aoSt", tag="aoSt")
        # O-projection weights double-buffer on the Act queue; the first two
        # batches are issued before attention so they stream underneath it.
        wop = es.enter_context(tc.tile_pool(name=pfx + "wop", bufs=2))
        wo_sbs = {}

        def load_wo(fc):
            wo_sb = wop.tile([128, 4, HB, 128], f8, name=f"wo{fc}", tag="wo")
            act.dma_start(wo_sb, wo_d[:, 4 * fc:4 * fc + 4, :, :])
            wo_sbs[fc] = wo_sb

        load_wo(0)
        load_wo(1)

        with tc.tile_pool(name=pfx + "kgp", bufs=2) as kgp, \
             tc.tile_pool(name=pfx + "vgp", bufs=2) as vgp, \
             tc.tile_pool(name=pfx + "ep", bufs=2) as ep, \
             tc.tile_pool(name=pfx + "pss", bufs=2, space="PSUM") as pssp, \
             tc.tile_pool(name=pfx + "av0", bufs=2, space="PSUM") as avp0, \
             tc.tile_pool(name=pfx + "av1", bufs=2, space="PSUM") as avp1, \
             tc.tile_pool(name=pfx + "dnp", bufs=2) as dnp:
            for g in range(KVH):
                kg_sb = kgp.tile([128, n_cores, s_pc], f32r, name=f"kg{g}",
                                 tag="kg")
                sync.dma_start(kg_sb, KG[:, :, g, :].rearrange("r d t -> d r t"))
                vgo = vgp.tile([128, KT, VW], f16, name=f"vg{g}", tag="vg")
                sync.dma_start(
                    vgo[:, :, 0:128],
                    VG.rearrange("r t p f -> p (r t) f")[:, :,
                                                         g * 128:(g + 1) * 128])
                vec.memset(vgo[:, :, 128:129], 1.0)
                for j in range(NREP):
                    hq = g * NREP + j
                    qh_ap = qTall[:, hq, :]
                    eT = ep.tile([128, KT, s_pc], bf16, name=f"eT{g}_{j}",
                                 tag="eT")
                    GRP = min(4, KT)
                    for kt0 in range(0, KT, GRP):
                        pss = pssp.tile([128, GRP * s_pc], f32,
                                        name=f"pss{g}_{j}_{kt0}", tag="pss")
                        for u in range(GRP):
                            kt = kt0 + u
                            pe.matmul(pss[:, u * s_pc:(u + 1) * s_pc],
                                      kg_sb[:, kt // NT,
                                            (kt % NT) * 128:
                                            (kt % NT) * 128 + 128],
                                      qh_ap, start=True, stop=True)
                        act.activation(
                            eT[:, kt0:kt0 + GRP, :].rearrange("p a b -> p (a b)"),
                            pss, AF.Exp, scale=SM_SCALE)
                    # attention output + softmax denominator in one pass:
                    # out[q, 0:128] = sum_k e[k,q] v[k,:], out[q, 128] = sum_k e[k,q]
                    avs = [avp0.tile([128, VW], f32, name=f"av0_{g}_{j}", tag="av0"),
                           avp1.tile([128, VW], f32, name=f"av1_{g}_{j}", tag="av1")]
                    for kt in range(KT):
                        for qh in range(2):
                            pe.matmul(avs[qh][:, 0:129],
                                      eT[:, kt, qh * 128:(qh + 1) * 128],
                                      vgo[:, kt, 0:129],
                                      start=(kt == 0), stop=(kt == KT - 1))
                    for qh in range(2):
                        dn = dnp.tile([128, 1], f32, name=f"dn{g}_{j}_{qh}",
                                      tag="dn")
                        vec.reciprocal(dn, avs[qh][:, 128:129])
                        vec.tensor_scalar_mul(aoSt[:, hq, qh, :],
                                              avs[qh][:, 0:128], dn)
                        tred = dnp.tile([128, 1], f32, name=f"tr{g}_{j}_{qh}",
                                        tag="tred")
                        vec.tensor_reduce(tred, aoSt[:, hq, qh, :], axis=AX.X,
                                          op=ALU.max, apply_absolute_value=True)
                        vec.tensor_tensor(racc[:, qh:qh + 1], racc[:, qh:qh + 1],
                                          tred, ALU.max)

        # ---------------- phase 6: re-quant + O projection ----------------
        with tc.tile_pool(name=pfx + "q2p", bufs=1) as q2p, \
             tc.tile_pool(name=pfx + "t1p", bufs=4) as t1p, \
             tc.tile_pool(name=pfx + "wop", bufs=2) as wop, \
             tc.tile_pool(name=pfx + "psy", bufs=4, space="PSUM") as psyp, \
             tc.tile_pool(name=pfx + "psb5", bufs=1, space="PSUM") as psb5, \
             tc.tile_pool(name=pfx + "yp", bufs=2) as yp:
            amc2 = q2p.tile([128, 2], f32, name="amc2", tag="amc2")
            vec.tensor_scalar_max(amc2, racc, 1e-5)
            r2 = q2p.tile([128, 2], f32, name="r2", tag="r2")
            vec.tensor_scalar_mul(r2, amc2, 1.0 / QB)
            inv2 = q2p.tile([128, 2], f32, name="inv2", tag="inv2")
            vec.reciprocal(inv2, amc2)
            sc2 = q2p.tile([128, 2], f32, name="sc2", tag="sc2")
            vec.tensor_scalar_mul(sc2, inv2, QB)
            for qh in range(2):
                sync.dma_start(r2_dram[0, qh * 128:(qh + 1) * 128], r2[:, qh])
            r2_row = q2p.tile([1, s_pc], f32, name="r2_row", tag="r2_row")
            sync.dma_start(r2_row, r2_dram[:])
            R2 = q2p.tile([128, s_pc], f32, name="R2", tag="R2")
            bcast_row(psb5, ones1, r2_row, R2, s_pc, "r2")

            # re-quant: ao2 = round(aoSt * sc2) as bf16 ints, [q, head, qh, f]
            ao2 = q2p.tile([128, QH, 2, 128], bf16, name="ao2", tag="ao2")
            for qh in range(2):
                for c in range(4):
                    sl = aoSt[:, c * 8:(c + 1) * 8, qh, :]
                    t1 = t1p.tile([128, 8, 128], f32, name=f"t1_{qh}_{c}",
                                  tag="t1")
                    eng = vec if (c % 2 == 0) else None
                    if eng is not None:
                        vec.tensor_scalar(t1, sl, sc2[:, qh:qh + 1], ROUND_C,
                                          ALU.mult, ALU.add)
                        vec.tensor_scalar_add(ao2[:, c * 8:(c + 1) * 8, qh, :],
                                              t1, -ROUND_C)
                    else:
                        act.activation(t1, sl, AF.Identity,
                                       bias=cpos, scale=sc2[:, qh:qh + 1])
                        act.activation(ao2[:, c * 8:(c + 1) * 8, qh, :], t1,
                                       AF.Identity, bias=cneg, scale=1.0)

            # one batched xbar transpose: ao2T[f, (head,qh), q] holds the
            # O-projection moving operand [hid, tok]
            ao2T = q2p.tile([128, 2 * QH, 128], bf16, name="ao2T", tag="ao2T")
            sync.dma_start(ao2T, ao2.rearrange("p a b c -> p (a b c)"),
                           transpose=True)
            if dbg_d is not None:
                sync.dma_start(dbg_d["daoSt"], aoSt)
                sync.dma_start(dbg_d["dao2T"], ao2T)

            for fc in range(HB // 4):
                if fc >= 2:
                    load_wo(fc)
                wo_sb = wo_sbs[fc]
                yT4 = yp.tile([128, 4, s_pc], f32, name=f"yT{fc}", tag="yT")
                for j in range(4):
                    f = fc * 4 + j
                    ps = psyp.tile([128, s_pc], f32, name=f"psy{f}", tag="psy")
                    for k in range(HB):
                        pe.matmul(ps, wo_sb[:, j, k, :],
                                  ao2T[:, 2 * k:2 * k + 2, :].rearrange(
                                      "p a b -> p (a b)"),
                                  start=(k == 0), stop=(k == HB - 1))
                    vec.scalar_tensor_tensor(yT4[:, j, :], ps, sw_o, R2,
                                             op0=ALU.mult, op1=ALU.mult)
                sync.dma_start(
                    y_d.rearrange("(a p) t -> p a t", p=128)[:, 4 * fc:4 * fc + 4, :],
                    yT4)


# ---------------------------------------------------------------------------
# host side
# ---------------------------------------------------------------------------

def _weight_quant_host(W):
    """Mimic reference _weight_quant: returns ternary m in {-1,0,1} and the
    effective dequant scale (1/scale) as fp32."""
    W = np.asarray(W, dtype=np.float32)
    mean_abs = np.float32(np.mean(np.abs(W), dtype=np.float64))
    clipped = np.maximum(mean_abs, np.float32(1e-5))
    scale = np.float32(1.0) / clipped
    m = np.clip(np.round(W * scale), -1.0, 1.0).astype(np.float32)
    sw = np.float32(1.0) / scale    # dequant scale applied after int matmul
    return m, float(sw)


def _prep_weights(Wq, Wk, Wv, Wo):
    mq, swq = _weight_quant_host(Wq)
    mk, swk = _weight_quant_host(Wk)
    mv, swv = _weight_quant_host(Wv)
    mo, swo = _weight_quant_host(Wo)

    def blocked(mT, fb):  # mT: [H, out] -> [128, fb, HB, 128]
        return np.ascontiguousarray(
            mT.reshape(HB, 128, fb, 128).transpose(1, 2, 0, 3)
        ).astype(np.float32)

    wqt = blocked(mq.T, QH)
    wkt = blocked(mk.T, KVH)
    wot = blocked(mo.T, HB)
    wvt = np.ascontiguousarray(
        mv.T.reshape(HB, 128, KVH * D).transpose(1, 0, 2)).astype(np.float32)
    f8np = mybir.dt.np(mybir.dt.float8e4)
    tof8 = lambda a: a.astype(f8np)
    return (tof8(wqt), tof8(wkt), tof8(wvt), tof8(wot),
            np.array([[swq, swk, swv, swo]], dtype=np.float32))


def _rope_tables(S):
    inv = (1.0 / (10000.0 ** (np.arange(0, D, 2, dtype=np.float32)
                              / np.float32(D)))).astype(np.float32)
    pos = np.arange(S, dtype=np.float32)
    fr = pos[:, None] * inv[None, :]          # [S, 64]
    emb = np.concatenate([fr, fr], axis=1)    # [S, D]
    cosT = np.cos(emb).T.astype(np.float32).copy()   # [D, S]
    sinT = np.sin(emb).T.astype(np.float32).copy()
    sinT[0:64, :] *= -1.0                      # sign baked for rotate-half
    return cosT, sinT


def _perm_matrix():
    # rot[i, t] = kdr[(i+64) % 128, t] as out = P^T @ kdr with stationary P:
    # P[c, i] = 1 iff c == (i+64) % 128
    P = np.zeros((128, 128), dtype=np.float32)
    for i in range(128):
        P[(i + 64) % 128, i] = 1.0
    return P


def _in_maps(inputs, n_cores=8, s_pc=256):
    hs = np.asarray(inputs["hidden_states"], dtype=np.float32)
    ln_w = np.asarray(inputs["ln_w"], dtype=np.float32).reshape(1, H)
    ln_b = np.asarray(inputs["ln_b"], dtype=np.float32).reshape(1, H)
    wqt, wkt, wvt, wot, wscal = _prep_weights(
        inputs["Wq"], inputs["Wk"], inputs["Wv"], inputs["Wo"])
    S = hs.shape[1]
    cosT, sinT = _rope_tables(S)
    P = _perm_matrix()
    maps = []
    for c in range(n_cores):
        sl = slice(c * s_pc, (c + 1) * s_pc)
        cos4 = np.ascontiguousarray(
            np.broadcast_to(cosT[:, None, sl], (D, 4, s_pc)))
        sin4 = np.ascontiguousarray(
            np.broadcast_to(sinT[:, None, sl], (D, 4, s_pc)))
        maps.append({
            "x": np.ascontiguousarray(hs[0, sl, :]),
            "lng": ln_w, "lnb": ln_b,
            "cos4": cos4, "sin4": sin4, "permM": P,
            "wqt": wqt, "wkt": wkt, "wvt": wvt, "wot": wot,
            "wscal": wscal,
        })
    return maps


_CACHED = {}


def _run(inputs, trace=False, n_cores=8, s_pc=256, return_res=False):
    from concourse.bass_utils import run_bass_kernel_spmd
    skip_gb = bool(
        np.allclose(np.asarray(inputs["ln_w"]), 1.0)
        and np.allclose(np.asarray(inputs["ln_b"]), 0.0))
    key = (n_cores, s_pc, skip_gb)
    if key not in _CACHED:
        _CACHED[key] = build(n_cores, s_pc, skip_gb=skip_gb)
    nc = _CACHED[key]
    maps = _in_maps(inputs, n_cores, s_pc)
    res = run_bass_kernel_spmd(nc, maps, list(range(n_cores)), trace=trace)
    parts = [res.results[c]["yT"].T for c in range(n_cores)]
    y = np.concatenate(parts, axis=0)[None, :, :].astype(np.float32)
    if return_res:
        return y, res.exec_time_ns, res
    return y, res.exec_time_ns


def kernel(**inputs):
    y, _ = _run(inputs, trace=False)
    return y
